# revision 21
# baseline (speedup 1.0000x reference)
"""Masked-softmax attention on 8 trn2 NeuronCores.

Reference computation (per batch b):
    att = q @ k                        # [n_q, n_k], k given pre-transposed [d, n_k]
    att = where(mask==0, -1e9, att)
    att = softmax(att, -1) / sqrt(d)
    out = (att @ v).T                  # returned [n_dv, n_q]

Sharding: data-parallel over batch: B=16 -> 2 batches per core x 8 cores.

Host-side, per batch, the key dimension is COMPACTED: masked-out keys
contribute exactly 0 to both the softmax numerator and denominator (the
reference's exp(-1e9 - anything) underflows to +0.0 in fp32), so we gather
only the unmasked columns of k / rows of v, padded up to a multiple of 128
(padding killed by the same -1e9 bias). With a Bernoulli(0.5) mask this
halves the contraction length. Exact, not an approximation.

Device-side plan (per batch). All matmul OPERANDS are bf16 (accumulation
stays fp32 in PSUM): on TRN2's PE both bf16 and f32r run 1 cycle/row at
512-wide moving, but bf16 halves every SBUF fetch and all input DMA, which
removes the SBUF-port contention between the PE's moving-operand stream
and the DVE's e-accumulation traffic. bf16 rounding lands ~1e-2 relative
on the output, inside the 2e-2 gate.

    - Work in the TRANSPOSED score layout S^T[k, q] (k on partitions):
        S^T tile [128k, 512q] = k_slice[d,128k]^T @ qT[d, 512q]  (2 d-chunk accum)
      `k` input [d, n_k] is directly the stationary operand; `q` is transposed
      host-side during sharding so qT[d, n_q] is directly the moving operand.
    - softmax is shift-invariant, so instead of the row max we subtract a
      CONSTANT shift (scores ~ N(0, d) with d=256 -> |s| < ~110 always;
      exp(s-shift) can't overflow and dominant terms can't underflow).
      Mask + shift fold into the scalar-engine exp as a per-partition bias:
        e[k, q] = exp(s + bias_k),  bias_k = -shift - 1e9*(1-mask_k)
    - out^T[dv, q] += v_tile[128k, dv_chunk]^T @ e   (v is directly stationary)
      z[dv, q]    += sixteens[128k, 128]^T @ e       (= 16Z in EVERY partition:
      the all-16s stationary matrix computes the row sum AND broadcasts it,
      folding in the post-softmax 1/sqrt(d)=1/16 scale)
    - out = out^T * (1/z) (DVE approx reciprocal) -> [dv, n_q], the required
      output layout.

DMA schedule (trace-derived, v2): ONLY the two HWDGE queues are used, and
every input DMA is posted up-front in strict priority order — the per-queue
FIFO then drains bytes in deadline order with no gating tricks and no
SWDGE (gpsimd) traffic competing for SDMA engines during the critical
fill. Profiling the v1 kernel showed the first real matmul waiting until
~13.8us because the k chunks were queued behind v and competing with a
1MB gpsimd queue; with the priority-FIFO fill the critical 576KB
(q stripe 0 + k chunk 0, split across both queues) lands ~9.5-10us.
  SP  queue: q0s0h0, k0c0h0, k0c1h0, v0c0, v0c1, q0s1-3h0, q0s1-3h1,
             q1h0, q1h1, k1h0, k1h1, v1  — then the per-stripe o0 output
             DMAs (each waits on its stripe's normalize, so they follow).
  ACT queue: q0s0h1, k0c0h1, bias, k0c1h1 — then per-stripe o1 outputs
             interleaved between exps.
v is pre-transposed host-side to partition-major [P, nkt*D] so its
transfers are fully contiguous. Outputs (bf16) go to a blocked DRAM
layout [NB, NQS, 2, P, QS] so each is one contiguous 1KB-row transfer;
the host reassembles + casts to f32.

gpsimd is NOT used at all: its tensor ops forced a Pool ucode library
swap (UNLOAD_LIB + 27KB LOAD_LIB) right in the final-stripe drain path
(~1.2us) plus a reload in the epilogue.

The inner loop is software-pipelined by FOUR k-tiles (O(t) emitted after
S(t+4), ps_s bufs=5 / ps_o bufs=1 / ps_z 1 = 8 PSUM banks): the in-order
PE queue then never reaches an O matmul before its ~0.9us S->exp chain
resolves. PE warmup matmuls (NWARM) bridge the preamble->first-input
window so the HAM clock gate ramps once and stays at K=8/8.
"""

import numpy as np
import ml_dtypes

import concourse.bacc as bacc
import concourse.mybir as mybir
import concourse.tile as tile
from concourse.bass_utils import run_bass_kernel_spmd

P = 128          # partitions
D = 256          # d == n_dv
S = 2048         # n_q
NB = 2           # batches per core
QS = 512         # q-stripe width (max matmul moving dim into one PSUM bank)
NQS = S // QS    # 4 q-stripes
NCORES = 8
SHIFT = 60.0     # constant softmax shift (see module docstring)
NWARM = 32       # PE warmup matmuls: bridge preamble (~7.6us) -> first
                 # input ready (~11.1us). Must leave NO idle gap before the
                 # real matmuls: the HAM clock gate only reaches K=8/8
                 # after ~3.4us of SUSTAINED PE activity, and a ~1.7us hole
                 # (measured, v2) kept it at half clock until 14.9us.

F32 = mybir.dt.float32
BF16 = mybir.dt.bfloat16
EXP = mybir.ActivationFunctionType.Exp
MULT = mybir.AluOpType.mult
ADD = mybir.AluOpType.add

BF16NP = ml_dtypes.bfloat16


def build(sk):
    """Build the per-core program. sk = compacted key length (mult of 128)."""
    from contextlib import ExitStack

    nkt = sk // P  # number of k-tiles
    nc = bacc.Bacc()
    qT = nc.declare_dram_parameter("qT", [NB, D, S], BF16, isOutput=False)
    kk = nc.declare_dram_parameter("k", [NB, D, sk], BF16, isOutput=False)
    # v pre-transposed host-side to partition-major [P, nkt*D]: v DMAs
    # become fully contiguous on both sides (4.6KB rows, ~2x throughput)
    vv = nc.declare_dram_parameter("v", [NB, P, (sk // P) * D], BF16, isOutput=False)
    # bias pre-packed host-side as [P, NB*128] so one 1KB-line transfer
    # moves both batches' bias columns
    bb = nc.declare_dram_parameter("bias", [P, NB * P], F32, isOutput=False)
    # blocked output layout: one contiguous [P, QS] block per
    # (batch, stripe, dv-half) so each output DMA moves 2KB rows; the host
    # reassembles [NB, D, S] with a cheap numpy transpose.
    out = nc.declare_dram_parameter("out", [NB, NQS, 2, P, QS], BF16, isOutput=True)

    def chunks(lo, hi, n):
        """Split [lo,hi) into n roughly-equal spans (empty spans dropped)."""
        step = max(1, (hi - lo + n - 1) // n)
        return [(a, min(a + step, hi)) for a in range(lo, hi, step)]

    with tile.TileContext(nc) as tc, ExitStack() as ctx:
        consts = ctx.enter_context(tc.tile_pool(name="consts", bufs=1))
        inp = ctx.enter_context(tc.tile_pool(name="inp", bufs=1))
        epool = ctx.enter_context(tc.tile_pool(name="e", bufs=8))
        opool = ctx.enter_context(tc.tile_pool(name="o", bufs=2))
        zpool = ctx.enter_context(tc.tile_pool(name="z", bufs=2))
        # 5 S banks + 2 O banks + 1 Z = 8. Single-buffered O is safe: the
        # normalize DVE ops of stripe s are emitted before stripe s+1's adds
        # on the in-order DVE, so the banks are free ~2.6us before stripe
        # s+1's first O matmul needs them.
        ps_s = ctx.enter_context(tc.tile_pool(name="ps_s", bufs=5, space="PSUM"))
        ps_o = ctx.enter_context(tc.tile_pool(name="ps_o", bufs=1, space="PSUM"))
        ps_z = ctx.enter_context(tc.tile_pool(name="ps_z", bufs=1, space="PSUM"))

        # memset packs the constant into the tile dtype's bits, so bf16
        # is written directly — no f32 staging, sixteens ready ~0.2us
        # after the DVE exits the entry barrier.
        sixteens = consts.tile([P, P], BF16)
        nc.vector.memset(sixteens, 16.0)

        # Warmup Exp: walrus attaches the implicit ACT table load to the
        # first Exp, which eats its sync-wait slots; give it a dep-free one
        # (also hides the ~1.3us table load under the input DMA fill).
        warm_in = consts.tile([P, 1], F32)
        nc.vector.memset(warm_in, 0.0)
        warm_out = consts.tile([P, 1], F32)
        nc.scalar.activation(warm_out, warm_in, EXP)

        # ---- all input tiles, both batches resident simultaneously
        kts = [
            [inp.tile([P, sk], BF16, tag=f"k{b}{c}", name=f"kt{b}{c}") for c in range(2)]
            for b in range(NB)
        ]
        qts = [
            [inp.tile([P, S], BF16, tag=f"q{b}{c}", name=f"qt{b}{c}") for c in range(2)]
            for b in range(NB)
        ]
        vts = [inp.tile([P, nkt, D], BF16, tag=f"v{b}", name=f"vt{b}") for b in range(NB)]
        bias_all = inp.tile([P, NB * P], F32, tag="bias", name="bias_all")
        biast = [bias_all[:, b * P : b * P + nkt] for b in range(NB)]

        kparts = chunks(0, nkt, 2)

        def q_span_dma(eng, b, s0, s1, c):
            eng.dma_start(
                out=qts[b][c][:, s0 * QS : s1 * QS],
                in_=qT[b, c * P : (c + 1) * P, s0 * QS : s1 * QS],
            )

        def v_chunk_dma(eng, b, t0, t1):
            eng.dma_start(
                out=vts[b][:, t0:t1, :],
                in_=vv[b, :, t0 * D : t1 * D],
            )

        def k_chunk_dma(eng, b, c, t0, t1):
            eng.dma_start(
                out=kts[b][c][:, t0 * P : t1 * P],
                in_=kk[b, c * P : (c + 1) * P, t0 * P : t1 * P],
            )

        # ---- ALL input DMAs posted up-front, priority-ordered; each
        # queue drains FIFO so order encodes the deadline schedule.
        # Measured (v2): the two HWDGE queues run ~symmetric ~115 GB/s
        # each while draining 1-1.25KB-row descriptors (SDMA engines
        # round-robin the queues per PACKET, so bytes/packet sets the
        # split — the 2-4KB-row bulk transfers late in each list speed
        # that queue up). No SWDGE: a third queue steals packet slots
        # from the critical fill (v1 measured the ACT queue at ~60GB/s
        # with gpsimd's 1MB queued). Deadlines (first MM ~11.4):
        #   k c0+q s0 by 11.4, bias 12.4, v t0-1 by 12.8, v t2-4 by
        #   14.7, k c1 by 16.9, v t5+ by 17.5, q s1 by 21.5, rest 28+.
        # SP queue:
        k_chunk_dma(nc.sync, 0, 0, *kparts[0])          # k0 c0 h0
        q_span_dma(nc.sync, 0, 0, 1, 0)                 # q0 s0 h0
        # q0 s0 h1 is split by column across the queues ~60/40: the SP
        # queue measures ~135 GB/s vs ACT's ~75-90, so an even half split
        # of the critical set leaves the first matmul gated on ACT.
        nc.sync.dma_start(
            out=qts[0][1][:, 0:QS // 2],
            in_=qT[0, P : 2 * P, 0:QS // 2],
        )
        v_chunk_dma(nc.sync, 0, 0, min(2, nkt))         # v0 t0-1 (first O tiles)
        if nkt > 2:
            v_chunk_dma(nc.sync, 0, 2, min(5, nkt))     # v0 t2-4
        if len(kparts) > 1:
            k_chunk_dma(nc.sync, 0, 0, *kparts[1])      # k0 c1 h0
        if nkt > 5:
            v_chunk_dma(nc.sync, 0, 5, nkt)             # v0 t5+
        q_span_dma(nc.sync, 0, 1, 2, 0)                 # q0 s1 h0
        q_span_dma(nc.sync, 0, 1, 2, 1)                 # q0 s1 h1
        for b in range(1, NB):                          # batch-1 k/v
            k_chunk_dma(nc.sync, b, 0, 0, nkt)
            k_chunk_dma(nc.sync, b, 1, 0, nkt)
            v_chunk_dma(nc.sync, b, 0, nkt)
        # ACT queue (the ACT engine must be free for exps by ~11.5us and
        # each post costs it ~0.7us, so only 4 posts go before the
        # compute loop; the bulk-q posts are emitted inside stripe (0,0)
        # between exps, by which point their FIFO position is harmless
        # and their 3-4KB rows drain fast).
        k_chunk_dma(nc.scalar, 0, 1, *kparts[0])        # k0 c0 h1
        nc.scalar.dma_start(
            out=qts[0][1][:, QS // 2 : QS],
            in_=qT[0, P : 2 * P, QS // 2 : QS],
        )                                               # q0 s0 h1 (tail cols)
        nc.scalar.dma_start(out=bias_all, in_=bb[:, :])  # bias
        if len(kparts) > 1:
            k_chunk_dma(nc.scalar, 0, 1, *kparts[1])    # k0 c1 h1

        # PE warmup: dep-free matmuls during the initial DMA fill so the HAM
        # clock gate ramps before the real matmuls start.
        for w in range(NWARM):
            wp = ps_s.tile([P, P], F32, tag="s", name=f"warm{w}")
            nc.tensor.matmul(wp, lhsT=sixteens, rhs=sixteens, start=True, stop=True)

        # ---- compute, one 512-wide q-stripe at a time
        for b in range(NB):
            for s in range(NQS):
                last_stripe = b == NB - 1 and s == NQS - 1
                qoff, qw = s * QS, QS
                qsl = slice(qoff, qoff + qw)
                op0 = ps_o.tile([P, QS], F32, tag="o0", name="op0")[:, :qw]
                op1 = ps_o.tile([P, QS], F32, tag="o1", name="op1")[:, :qw]
                zp = ps_z.tile([P, QS], F32, tag="z", name="zp")[:, :qw]
                acc_e = None

                def s_exp(t):
                    """Emit the S matmul pair + exp for k-tile t; return e."""
                    ksl = slice(t * P, (t + 1) * P)
                    sp = ps_s.tile([P, QS], F32, tag="s", name="sp")[:, :qw]
                    nc.tensor.matmul(
                        sp, lhsT=kts[b][0][:, ksl], rhs=qts[b][0][:, qsl],
                        start=True, stop=False,
                    )
                    nc.tensor.matmul(
                        sp, lhsT=kts[b][1][:, ksl], rhs=qts[b][1][:, qsl],
                        start=False, stop=True,
                    )
                    e = epool.tile([P, QS], BF16, tag="e", name="e")[:, :qw]
                    nc.scalar.activation(e, sp, EXP, bias=biast[b][:, t : t + 1])
                    return e

                # Software-pipelined by FOUR tiles: O(t) is emitted after
                # S(t+4). Two tiles (~1.7us) covers the ~0.9us S->exp
                # latency; the extra depth lets the PE scoreboard keep
                # running S matmuls ahead while early v chunks are still in
                # flight. ps_s bufs=5 holds sp(t)..sp(t+4).
                DEPTH = 4
                pipe = [s_exp(tt) for tt in range(min(DEPTH, nkt))]
                # deferred bulk-q posts on the ACT queue: emitted between
                # exps so the ACT engine's ~0.7us/post cost lands in its
                # per-stripe slack; by now their FIFO position is behind
                # all critical transfers and their 3-4KB rows drain fast.
                if b == 0 and s == 0:
                    q_span_dma(nc.scalar, 0, 2, NQS, 0)   # q0 s2-3 h0
                    q_span_dma(nc.scalar, 0, 2, NQS, 1)   # q0 s2-3 h1
                if b == 0 and s == 1:
                    for b2 in range(1, NB):
                        q_span_dma(nc.scalar, b2, 0, NQS, 0)  # q1 h0 full
                        q_span_dma(nc.scalar, b2, 0, NQS, 1)  # q1 h1 full
                for t in range(nkt):
                    e = pipe.pop(0)
                    if t + DEPTH < nkt:
                        pipe.append(s_exp(t + DEPTH))
                    first, last = t == 0, t == nkt - 1
                    # Z: a running DVE accumulator sums ALL the stripe's
                    # e-tiles so only ONE Z matmul runs, and on the final
                    # tile that Z is emitted BEFORE the O pair (its DVE add
                    # chain resolved ~2 tiles ago thanks to the pipeline):
                    # the normalize reciprocal then overlaps the stripe's
                    # last O matmuls, so the whole recip+mult chain fits
                    # inside the next stripe's 8-matmul prologue window and
                    # the single-buffered O banks are free in time.
                    if acc_e is None:
                        acc_e = e
                    else:
                        na = epool.tile([P, QS], BF16, tag="ep", name="na")[:, :qw]
                        nc.vector.tensor_tensor(na, acc_e, e, ADD)
                        acc_e = na
                    if last:
                        nc.tensor.matmul(
                            zp, lhsT=sixteens, rhs=acc_e, start=True, stop=True,
                        )
                        acc_e = None
                    nc.tensor.matmul(
                        op0, lhsT=vts[b][:, t, 0:P], rhs=e, start=first, stop=last,
                    )
                    nc.tensor.matmul(
                        op1, lhsT=vts[b][:, t, P : 2 * P], rhs=e, start=first, stop=last,
                    )
                # normalize: out = out_unnorm * (1/(16Z)); zp already holds
                # 16Z in every partition. ~18-bit reciprocal, 5x faster than
                # exact; z is far from denorm/inf so approx edge cases can't
                # hit. Processed in chunks so the tail (recip -> mult -> DMA)
                # pipelines; the final stripe uses finer chunks to shorten
                # the drain.
                zbs = zpool.tile([P, QS], F32, tag="zbs", name="zbs")[:, :qw]
                o0 = opool.tile([P, QS], BF16, tag="so0", name="o0")[:, :qw]
                o1 = opool.tile([P, QS], BF16, tag="so1", name="o1")[:, :qw]
                if last_stripe:
                    # Drain: the recip runs concurrently with the final O
                    # matmuls (Z was issued before them); ACT then copies
                    # op1 out of PSUM while the DVE multiplies o0 straight
                    # from PSUM, and o1's multiply runs from SBUF. One DVE
                    # op per output — PSUM-touching DVE ops are latency-
                    # bound at ~0.42us regardless of width, so fewer is
                    # faster (measured: 4 chunked mults cost 1.66us serial).
                    o1s = opool.tile([P, QS], F32, tag="so1c", name="o1s")[:, :qw]
                    COPY = mybir.ActivationFunctionType.Copy
                    nc.vector.reciprocal_approx_fast(out=zbs, in_=zp)
                    nc.scalar.activation(o1s, op1, COPY)
                    nc.vector.tensor_tensor(o0, op0, zbs, MULT)
                    nc.sync.dma_start(out=out[b, s, 0], in_=o0)
                    nc.vector.tensor_tensor(o1, o1s, zbs, MULT)
                    # final output split across both queues: halves move in
                    # parallel, the binding receipt fires ~0.5us earlier
                    nc.scalar.dma_start(
                        out=out[b, s, 1, :, 0 : QS // 2], in_=o1[:, 0 : QS // 2]
                    )
                    nc.sync.dma_start(
                        out=out[b, s, 1, :, QS // 2 : QS], in_=o1[:, QS // 2 : QS]
                    )
                else:
                    nc.vector.reciprocal_approx_fast(out=zbs, in_=zp)
                    nc.vector.tensor_tensor(o0, op0, zbs, MULT)
                    nc.sync.dma_start(out=out[b, s, 0], in_=o0)
                    nc.vector.tensor_tensor(o1, op1, zbs, MULT)
                    nc.scalar.dma_start(out=out[b, s, 1], in_=o1)

    return nc


def build_balanced(slot_nkts, bufmap, nbuf):
    """Stripe-balanced program: 8 independent 512-wide q-stripe SLOTS per
    core, slot j contracting over slot_nkts[j] k-tiles of kv-buffer
    bufmap[j]. With the measured mask (10 batches at 9 k-tiles, 6 at 8),
    slots [9,9,9,9,9,8,8,8] give every core 69 k-tile-units instead of the
    batch-pair scheme's 72 (the 10 nine-batches pigeonhole at least one
    core into 9+9): ~2.7us less PE time on the slowest core.
    """
    from contextlib import ExitStack

    nmax = max(slot_nkts)
    SK = nmax * P
    bufnkt = [max(slot_nkts[j] for j in range(8) if bufmap[j] == bf) for bf in range(nbuf)]
    nc = bacc.Bacc()
    # q packed slot-minor [P, 8*1024] so the bulk (slots 2-7) moves as ONE
    # contiguous 12KB-row transfer — measured, 1KB-row transfers run the
    # HWDGE queues at only ~115 GB/s while 4KB+ rows more than double that
    qs = nc.declare_dram_parameter("qs", [P, 8 * 2 * QS], BF16, isOutput=False)
    kk = nc.declare_dram_parameter("k", [nbuf, D, SK], BF16, isOutput=False)
    vv = nc.declare_dram_parameter("v", [nbuf, P, nmax * D], BF16, isOutput=False)
    bb = nc.declare_dram_parameter("bias", [P, nbuf * 32], F32, isOutput=False)
    out = nc.declare_dram_parameter("out", [8, 2, P, QS], BF16, isOutput=True)

    def chunks(lo, hi, n):
        step = max(1, (hi - lo + n - 1) // n)
        return [(a, min(a + step, hi)) for a in range(lo, hi, step)]

    with tile.TileContext(nc) as tc, ExitStack() as ctx:
        consts = ctx.enter_context(tc.tile_pool(name="consts", bufs=1))
        inp = ctx.enter_context(tc.tile_pool(name="inp", bufs=1))
        epool = ctx.enter_context(tc.tile_pool(name="e", bufs=8))
        # o bufs=3: the o0 output DMAs ride the SP queue BEHIND the bulk
        # input bytes and can land ~15us after their stripe; slot j+3's
        # buffer-reuse dependency gives them that slack
        opool = ctx.enter_context(tc.tile_pool(name="o", bufs=3))
        zpool = ctx.enter_context(tc.tile_pool(name="z", bufs=2))
        ps_s = ctx.enter_context(tc.tile_pool(name="ps_s", bufs=5, space="PSUM"))
        ps_o = ctx.enter_context(tc.tile_pool(name="ps_o", bufs=1, space="PSUM"))
        ps_z = ctx.enter_context(tc.tile_pool(name="ps_z", bufs=1, space="PSUM"))

        sixteens = consts.tile([P, P], BF16)
        nc.vector.memset(sixteens, 16.0)
        warm_in = consts.tile([P, 1], F32)
        nc.vector.memset(warm_in, 0.0)
        warm_out = consts.tile([P, 1], F32)
        nc.scalar.activation(warm_out, warm_in, EXP)

        kts = [
            [inp.tile([P, bufnkt[bf] * P], BF16, tag=f"k{bf}{c}", name=f"kt{bf}{c}")
             for c in range(2)]
            for bf in range(nbuf)
        ]
        qbig = inp.tile([P, 8 * 2 * QS], BF16, tag="qs", name="qbig")
        vts = [
            inp.tile([P, bufnkt[bf], D], BF16, tag=f"v{bf}", name=f"vt{bf}")
            for bf in range(nbuf)
        ]
        bias_all = inp.tile([P, nbuf * 32], F32, tag="bias", name="bias_all")
        biast = [bias_all[:, bf * 32 : bf * 32 + bufnkt[bf]] for bf in range(nbuf)]

        def q_dma(eng, c0, c1):
            eng.dma_start(out=qbig[:, c0:c1], in_=qs[:, c0:c1])

        def v_dma(eng, bf, t0, t1):
            eng.dma_start(out=vts[bf][:, t0:t1, :], in_=vv[bf, :, t0 * D : t1 * D])

        def k_dma(eng, bf, c, t0, t1):
            eng.dma_start(
                out=kts[bf][c][:, t0 * P : t1 * P],
                in_=kk[bf, c * P : (c + 1) * P, t0 * P : t1 * P],
            )

        nk0 = bufnkt[bufmap[0]]
        kparts = chunks(0, nk0, 2)
        # SP queue (priority FIFO; see build() for the rate model)
        k_dma(nc.sync, 0, 0, *kparts[0])                # k b0 c0 h0
        q_dma(nc.sync, 0, QS)                           # q s0 h0
        q_dma(nc.sync, QS, QS + QS // 2)                # q s0 h1 head cols
        v_dma(nc.sync, 0, 0, min(2, nk0))               # v b0 t0-1
        if nk0 > 2:
            v_dma(nc.sync, 0, 2, min(5, nk0))           # v b0 t2-4
        if len(kparts) > 1:
            k_dma(nc.sync, 0, 0, *kparts[1])            # k b0 c1 h0
        if nk0 > 5:
            v_dma(nc.sync, 0, 5, nk0)                   # v b0 t5+
        q_dma(nc.sync, 2 * QS, 4 * QS)                  # q slot 1
        q_dma(nc.sync, 4 * QS, 16 * QS)                 # q slots 2-7, one post
        for bf in range(1, nbuf):                       # remaining kv buffers
            k_dma(nc.sync, bf, 0, 0, bufnkt[bf])
            k_dma(nc.sync, bf, 1, 0, bufnkt[bf])
            v_dma(nc.sync, bf, 0, bufnkt[bf])
        # ACT queue: ONLY these four posts run on the ACT engine — every
        # extra post there delays exps by ~0.6us (measured: two deferred
        # posts cost a 2.8us exp wait / ~1.1us PE stall)
        k_dma(nc.scalar, 0, 1, *kparts[0])              # k b0 c0 h1
        q_dma(nc.scalar, QS + QS // 2, 2 * QS)          # q s0 h1 tail cols
        nc.scalar.dma_start(out=bias_all, in_=bb[:, :])  # bias
        if len(kparts) > 1:
            k_dma(nc.scalar, 0, 1, *kparts[1])          # k b0 c1 h1

        for w in range(NWARM):
            wp = ps_s.tile([P, P], F32, tag="s", name=f"warm{w}")
            nc.tensor.matmul(wp, lhsT=sixteens, rhs=sixteens, start=True, stop=True)

        for j in range(8):
            nkt = slot_nkts[j]
            bf = bufmap[j]
            last_slot = j == 7
            op0 = ps_o.tile([P, QS], F32, tag="o0", name="op0")
            op1 = ps_o.tile([P, QS], F32, tag="o1", name="op1")
            zp = ps_z.tile([P, QS], F32, tag="z", name="zp")
            acc_e = None

            qh0 = qbig[:, j * 2 * QS : j * 2 * QS + QS]
            qh1 = qbig[:, j * 2 * QS + QS : (j + 1) * 2 * QS]

            def s_exp(t):
                ksl = slice(t * P, (t + 1) * P)
                sp = ps_s.tile([P, QS], F32, tag="s", name="sp")
                nc.tensor.matmul(
                    sp, lhsT=kts[bf][0][:, ksl], rhs=qh0,
                    start=True, stop=False,
                )
                nc.tensor.matmul(
                    sp, lhsT=kts[bf][1][:, ksl], rhs=qh1,
                    start=False, stop=True,
                )
                e = epool.tile([P, QS], BF16, tag="e", name="e")
                nc.scalar.activation(e, sp, EXP, bias=biast[bf][:, t : t + 1])
                return e

            DEPTH = 4
            pipe = [s_exp(tt) for tt in range(min(DEPTH, nkt))]
            for t in range(nkt):
                e = pipe.pop(0)
                if t + DEPTH < nkt:
                    pipe.append(s_exp(t + DEPTH))
                first, last = t == 0, t == nkt - 1
                if acc_e is None:
                    acc_e = e
                else:
                    na = epool.tile([P, QS], BF16, tag="ep", name="na")
                    nc.vector.tensor_tensor(na, acc_e, e, ADD)
                    acc_e = na
                if last:
                    nc.tensor.matmul(zp, lhsT=sixteens, rhs=acc_e, start=True, stop=True)
                    acc_e = None
                nc.tensor.matmul(
                    op0, lhsT=vts[bf][:, t, 0:P], rhs=e, start=first, stop=last,
                )
                nc.tensor.matmul(
                    op1, lhsT=vts[bf][:, t, P : 2 * P], rhs=e, start=first, stop=last,
                )
            zbs = zpool.tile([P, QS], F32, tag="zbs", name="zbs")
            o0 = opool.tile([P, QS], BF16, tag="so0", name="o0")
            o1 = opool.tile([P, QS], BF16, tag="so1", name="o1")
            if last_slot:
                o1s = opool.tile([P, QS], F32, tag="so1c", name="o1s")
                COPY = mybir.ActivationFunctionType.Copy
                nc.vector.reciprocal_approx_fast(out=zbs, in_=zp)
                nc.scalar.activation(o1s, op1, COPY)
                nc.vector.tensor_tensor(o0, op0, zbs, MULT)
                nc.sync.dma_start(out=out[j, 0], in_=o0)
                nc.vector.tensor_tensor(o1, o1s, zbs, MULT)
                nc.scalar.dma_start(
                    out=out[j, 1, :, 0 : QS // 2], in_=o1[:, 0 : QS // 2]
                )
                nc.sync.dma_start(
                    out=out[j, 1, :, QS // 2 : QS], in_=o1[:, QS // 2 : QS]
                )
            else:
                nc.vector.reciprocal_approx_fast(out=zbs, in_=zp)
                nc.vector.tensor_tensor(o0, op0, zbs, MULT)
                nc.sync.dma_start(out=out[j, 0], in_=o0)
                nc.vector.tensor_tensor(o1, op1, zbs, MULT)
                nc.scalar.dma_start(out=out[j, 1], in_=o1)

    return nc


def make_in_maps_balanced(q, k, v, mask):
    """Stripe-balanced sharding (see build_balanced). Returns None when the
    mask's per-batch k-tile counts don't fit the {9-tiles: 10, 8-tiles: 6}
    pattern this packing is built for."""
    q = np.asarray(q, dtype=np.float32)
    k = np.asarray(k, dtype=np.float32)
    v = np.asarray(v, dtype=np.float32)
    mask = np.asarray(mask, dtype=np.int32).reshape(len(q), -1)
    B = len(q)
    if B != 16:
        return None
    idxs = [np.nonzero(mask[b])[0] for b in range(B)]
    nktb = [max(1, (len(ix) + P - 1) // P) for ix in idxs]
    order = sorted(range(B), key=lambda b: -len(idxs[b]))
    nine = [b for b in order if nktb[b] == 9]
    eight = [b for b in order if nktb[b] == 8]
    if len(nine) != 10 or len(eight) != 6:
        return None
    slot_nkts = [9, 9, 9, 9, 9, 8, 8, 8]
    bufmap = [0, 0, 0, 0, 1, 2, 2, 3]
    nbuf = 4
    nmax = 9
    SK = nmax * P

    # per-batch compacted k / v / bias, padded to 9 tiles
    kg = np.zeros((B, D, SK), dtype=np.float32)
    vgt = np.zeros((B, P, nmax * D), dtype=np.float32)
    bg = np.full((B, SK), -1.0e9, dtype=np.float32)
    for b in range(B):
        ix = idxs[b]
        kg[b, :, : len(ix)] = k[b][:, ix]
        vg = np.zeros((SK, D), dtype=np.float32)
        vg[: len(ix)] = v[b][ix]
        vgt[b] = vg.reshape(nmax, P, D).transpose(1, 0, 2).reshape(P, -1)
        bg[b, : len(ix)] = -SHIFT
    bgt = bg.reshape(B, nmax, P).transpose(0, 2, 1)  # [B, P, 9]
    qT = np.transpose(q, (0, 2, 1))  # [B, D, S]

    in_maps = []
    slot_maps = []
    for i in range(NCORES):
        bufs = [nine[i], nine[8 + i // 4], eight[i // 2], eight[4 + i // 4]]
        slots = [
            (bufs[0], 0), (bufs[0], 1), (bufs[0], 2), (bufs[0], 3),
            (bufs[1], i % 4),
            (bufs[2], 2 * (i % 2)), (bufs[2], 2 * (i % 2) + 1),
            (bufs[3], i % 4),
        ]
        # slot-minor q pack [P, 8*1024]: slots 2-7 then move as ONE
        # contiguous 12KB-row transfer
        qsl = np.zeros((P, 8 * 2 * QS), dtype=np.float32)
        for j, (b, s) in enumerate(slots):
            sl = slice(s * QS, (s + 1) * QS)
            qsl[:, j * 2 * QS : j * 2 * QS + QS] = qT[b, 0:P, sl]
            qsl[:, j * 2 * QS + QS : (j + 1) * 2 * QS] = qT[b, P : 2 * P, sl]
        bias = np.full((P, nbuf * 32), -1.0e9, dtype=np.float32)
        for bf in range(nbuf):
            bias[:, bf * 32 : bf * 32 + nmax] = bgt[bufs[bf]]
        in_maps.append(
            {
                "qs": np.ascontiguousarray(qsl.astype(BF16NP)),
                "k": np.ascontiguousarray(kg[bufs].astype(BF16NP)),
                "v": np.ascontiguousarray(vgt[bufs].astype(BF16NP)),
                "bias": np.ascontiguousarray(bias),
            }
        )
        slot_maps.append(slots)
    return in_maps, slot_nkts, bufmap, nbuf, slot_maps


def make_in_maps(q, k, v, mask):
    """Shard over batch; transpose q; compact the key dim to unmasked keys."""
    q = np.asarray(q, dtype=np.float32)
    k = np.asarray(k, dtype=np.float32)
    v = np.asarray(v, dtype=np.float32)
    mask = np.asarray(mask, dtype=np.int32).reshape(len(q), -1)

    B = len(q)
    idxs = [np.nonzero(mask[b])[0] for b in range(B)]
    n_eff = max((len(ix) for ix in idxs), default=1)
    sk = max(P, ((n_eff + P - 1) // P) * P)  # padded compacted key length

    kg = np.zeros((B, D, sk), dtype=np.float32)
    vg = np.zeros((B, sk, D), dtype=np.float32)
    # exp bias: -SHIFT for real keys, -1e9 for padding (kills it exactly),
    # laid out [P, sk//P] partition-major to match the k-tile slicing
    bg = np.full((B, sk), -1.0e9, dtype=np.float32)
    for b in range(B):
        ix = idxs[b]
        kg[b, :, : len(ix)] = k[b][:, ix]
        vg[b, : len(ix)] = v[b][ix]
        bg[b, : len(ix)] = -SHIFT
    bgt = bg.reshape(B, sk // P, P).transpose(0, 2, 1)  # [B, P, nkt]
    # v partition-major: vgt[b, p, t*D+d] = vg[b, t*128+p, d]
    vgt = vg.reshape(B, sk // P, P, D).transpose(0, 2, 1, 3).reshape(B, P, -1)
    bgp = np.zeros((B, P, P), dtype=np.float32)  # rows padded to 512B lines
    bgp[:, :, : sk // P] = bgt
    # pack per-core as [P, NB*128]: core i gets batches i*NB..i*NB+NB-1
    bgq = bgp.transpose(1, 0, 2).reshape(P, B * P)

    in_maps = []
    for i in range(NCORES):
        sl = slice(i * NB, (i + 1) * NB)
        in_maps.append(
            {
                "qT": np.ascontiguousarray(
                    np.transpose(q[sl], (0, 2, 1)).astype(BF16NP)
                ),
                "k": np.ascontiguousarray(kg[sl].astype(BF16NP)),
                "v": np.ascontiguousarray(vgt[sl].astype(BF16NP)),
                "bias": np.ascontiguousarray(
                    bgq[:, i * NB * P : (i + 1) * NB * P]
                ),
            }
        )
    return in_maps, sk


def run(q, k, v, mask, **kwargs):
    bal = make_in_maps_balanced(q, k, v, mask)
    if bal is not None:
        in_maps, slot_nkts, bufmap, nbuf, slot_maps = bal
        nc = build_balanced(slot_nkts, bufmap, nbuf)
        nc.finalize()
        res = run_bass_kernel_spmd(nc, in_maps, list(range(NCORES)), **kwargs)
        B = len(slot_maps) * 0 + 16
        out = np.zeros((B, D, S), dtype=np.float32)
        for i, r in enumerate(res.results):
            blk = np.asarray(r["out"], dtype=np.float32)  # [8, 2, P, QS]
            for j, (b, s) in enumerate(slot_maps[i]):
                sl = slice(s * QS, (s + 1) * QS)
                out[b, 0:P, sl] = blk[j, 0]
                out[b, P : 2 * P, sl] = blk[j, 1]
        return out, res

    in_maps, sk = make_in_maps(q, k, v, mask)
    nc = build(sk)
    nc.finalize()  # run the Bacc pass pipeline (reg alloc, wait splitting)
    res = run_bass_kernel_spmd(nc, in_maps, list(range(NCORES)), **kwargs)
    # device layout [NB, NQS, 2, P, QS] -> [NB, D, S]
    out = np.concatenate(
        [
            r["out"].transpose(0, 2, 3, 1, 4).reshape(NB, D, S)
            for r in res.results
        ],
        axis=0,
    ).astype(np.float32)
    return out, res


def kernel(q, k, v, mask):
    out, _ = run(q, k, v, mask)
    return out


# revision 30
# speedup vs baseline: 1.0131x; 1.0131x over previous
"""Masked-softmax attention on 8 trn2 NeuronCores.

Reference computation (per batch b):
    att = q @ k                        # [n_q, n_k], k given pre-transposed [d, n_k]
    att = where(mask==0, -1e9, att)
    att = softmax(att, -1) / sqrt(d)
    out = (att @ v).T                  # returned [n_dv, n_q]

Sharding: data-parallel over batch: B=16 -> 2 batches per core x 8 cores.

Host-side, per batch, the key dimension is COMPACTED: masked-out keys
contribute exactly 0 to both the softmax numerator and denominator (the
reference's exp(-1e9 - anything) underflows to +0.0 in fp32), so we gather
only the unmasked columns of k / rows of v, padded up to a multiple of 128
(padding killed by the same -1e9 bias). With a Bernoulli(0.5) mask this
halves the contraction length. Exact, not an approximation.

Device-side plan (per batch). All matmul OPERANDS are bf16 (accumulation
stays fp32 in PSUM): on TRN2's PE both bf16 and f32r run 1 cycle/row at
512-wide moving, but bf16 halves every SBUF fetch and all input DMA, which
removes the SBUF-port contention between the PE's moving-operand stream
and the DVE's e-accumulation traffic. bf16 rounding lands ~1e-2 relative
on the output, inside the 2e-2 gate.

    - Work in the TRANSPOSED score layout S^T[k, q] (k on partitions):
        S^T tile [128k, 512q] = k_slice[d,128k]^T @ qT[d, 512q]  (2 d-chunk accum)
      `k` input [d, n_k] is directly the stationary operand; `q` is transposed
      host-side during sharding so qT[d, n_q] is directly the moving operand.
    - softmax is shift-invariant, so instead of the row max we subtract a
      CONSTANT shift (scores ~ N(0, d) with d=256 -> |s| < ~110 always;
      exp(s-shift) can't overflow and dominant terms can't underflow).
      Mask + shift fold into the scalar-engine exp as a per-partition bias:
        e[k, q] = exp(s + bias_k),  bias_k = -shift - 1e9*(1-mask_k)
    - out^T[dv, q] += v_tile[128k, dv_chunk]^T @ e   (v is directly stationary)
      z[dv, q]    += sixteens[128k, 128]^T @ e       (= 16Z in EVERY partition:
      the all-16s stationary matrix computes the row sum AND broadcasts it,
      folding in the post-softmax 1/sqrt(d)=1/16 scale)
    - out = out^T * (1/z) (DVE approx reciprocal) -> [dv, n_q], the required
      output layout.

DMA schedule (trace-derived, v2): ONLY the two HWDGE queues are used, and
every input DMA is posted up-front in strict priority order — the per-queue
FIFO then drains bytes in deadline order with no gating tricks and no
SWDGE (gpsimd) traffic competing for SDMA engines during the critical
fill. Profiling the v1 kernel showed the first real matmul waiting until
~13.8us because the k chunks were queued behind v and competing with a
1MB gpsimd queue; with the priority-FIFO fill the critical 576KB
(q stripe 0 + k chunk 0, split across both queues) lands ~9.5-10us.
  SP  queue: q0s0h0, k0c0h0, k0c1h0, v0c0, v0c1, q0s1-3h0, q0s1-3h1,
             q1h0, q1h1, k1h0, k1h1, v1  — then the per-stripe o0 output
             DMAs (each waits on its stripe's normalize, so they follow).
  ACT queue: q0s0h1, k0c0h1, bias, k0c1h1 — then per-stripe o1 outputs
             interleaved between exps.
v is pre-transposed host-side to partition-major [P, nkt*D] so its
transfers are fully contiguous. Outputs (bf16) go to a blocked DRAM
layout [NB, NQS, 2, P, QS] so each is one contiguous 1KB-row transfer;
the host reassembles + casts to f32.

gpsimd is NOT used at all: its tensor ops forced a Pool ucode library
swap (UNLOAD_LIB + 27KB LOAD_LIB) right in the final-stripe drain path
(~1.2us) plus a reload in the epilogue.

The inner loop is software-pipelined by FOUR k-tiles (O(t) emitted after
S(t+4), ps_s bufs=5 / ps_o bufs=1 / ps_z 1 = 8 PSUM banks): the in-order
PE queue then never reaches an O matmul before its ~0.9us S->exp chain
resolves. PE warmup matmuls (NWARM) bridge the preamble->first-input
window so the HAM clock gate ramps once and stays at K=8/8.
"""

import numpy as np
import ml_dtypes

import concourse.bacc as bacc
import concourse.mybir as mybir
import concourse.tile as tile
from concourse.bass_utils import run_bass_kernel_spmd

P = 128          # partitions
D = 256          # d == n_dv
S = 2048         # n_q
NB = 2           # batches per core
QS = 512         # q-stripe width (max matmul moving dim into one PSUM bank)
NQS = S // QS    # 4 q-stripes
NCORES = 8
SHIFT = 60.0     # constant softmax shift (see module docstring)
NWARM = 38       # PE warmup matmuls: bridge preamble (~7.6us) -> first
                 # input ready (~11.1us). Must leave NO idle gap before the
                 # real matmuls: the HAM clock gate only reaches K=8/8
                 # after ~3.4us of SUSTAINED PE activity, and a ~1.7us hole
                 # (measured, v2) kept it at half clock until 14.9us.

F32 = mybir.dt.float32
BF16 = mybir.dt.bfloat16
EXP = mybir.ActivationFunctionType.Exp
MULT = mybir.AluOpType.mult
ADD = mybir.AluOpType.add

BF16NP = ml_dtypes.bfloat16


def build(sk):
    """Build the per-core program. sk = compacted key length (mult of 128)."""
    from contextlib import ExitStack

    nkt = sk // P  # number of k-tiles
    nc = bacc.Bacc()
    qT = nc.declare_dram_parameter("qT", [NB, D, S], BF16, isOutput=False)
    kk = nc.declare_dram_parameter("k", [NB, D, sk], BF16, isOutput=False)
    # v pre-transposed host-side to partition-major [P, nkt*D]: v DMAs
    # become fully contiguous on both sides (4.6KB rows, ~2x throughput)
    vv = nc.declare_dram_parameter("v", [NB, P, (sk // P) * D], BF16, isOutput=False)
    # bias pre-packed host-side as [P, NB*128] so one 1KB-line transfer
    # moves both batches' bias columns
    bb = nc.declare_dram_parameter("bias", [P, NB * P], F32, isOutput=False)
    # blocked output layout: one contiguous [P, QS] block per
    # (batch, stripe, dv-half) so each output DMA moves 2KB rows; the host
    # reassembles [NB, D, S] with a cheap numpy transpose.
    out = nc.declare_dram_parameter("out", [NB, NQS, 2, P, QS], BF16, isOutput=True)

    def chunks(lo, hi, n):
        """Split [lo,hi) into n roughly-equal spans (empty spans dropped)."""
        step = max(1, (hi - lo + n - 1) // n)
        return [(a, min(a + step, hi)) for a in range(lo, hi, step)]

    with tile.TileContext(nc) as tc, ExitStack() as ctx:
        consts = ctx.enter_context(tc.tile_pool(name="consts", bufs=1))
        inp = ctx.enter_context(tc.tile_pool(name="inp", bufs=1))
        epool = ctx.enter_context(tc.tile_pool(name="e", bufs=8))
        opool = ctx.enter_context(tc.tile_pool(name="o", bufs=2))
        zpool = ctx.enter_context(tc.tile_pool(name="z", bufs=2))
        # 5 S banks + 2 O banks + 1 Z = 8. Single-buffered O is safe: the
        # normalize DVE ops of stripe s are emitted before stripe s+1's adds
        # on the in-order DVE, so the banks are free ~2.6us before stripe
        # s+1's first O matmul needs them.
        ps_s = ctx.enter_context(tc.tile_pool(name="ps_s", bufs=5, space="PSUM"))
        ps_o = ctx.enter_context(tc.tile_pool(name="ps_o", bufs=1, space="PSUM"))
        ps_z = ctx.enter_context(tc.tile_pool(name="ps_z", bufs=1, space="PSUM"))

        # memset packs the constant into the tile dtype's bits, so bf16
        # is written directly — no f32 staging, sixteens ready ~0.2us
        # after the DVE exits the entry barrier.
        sixteens = consts.tile([P, P], BF16)
        nc.vector.memset(sixteens, 16.0)

        # Warmup Exp: walrus attaches the implicit ACT table load to the
        # first Exp, which eats its sync-wait slots; give it a dep-free one
        # (also hides the ~1.3us table load under the input DMA fill).
        warm_in = consts.tile([P, 1], F32)
        nc.vector.memset(warm_in, 0.0)
        warm_out = consts.tile([P, 1], F32)
        nc.scalar.activation(warm_out, warm_in, EXP)

        # ---- all input tiles, both batches resident simultaneously
        kts = [
            [inp.tile([P, sk], BF16, tag=f"k{b}{c}", name=f"kt{b}{c}") for c in range(2)]
            for b in range(NB)
        ]
        qts = [
            [inp.tile([P, S], BF16, tag=f"q{b}{c}", name=f"qt{b}{c}") for c in range(2)]
            for b in range(NB)
        ]
        vts = [inp.tile([P, nkt, D], BF16, tag=f"v{b}", name=f"vt{b}") for b in range(NB)]
        bias_all = inp.tile([P, NB * P], F32, tag="bias", name="bias_all")
        biast = [bias_all[:, b * P : b * P + nkt] for b in range(NB)]

        kparts = chunks(0, nkt, 2)

        def q_span_dma(eng, b, s0, s1, c):
            eng.dma_start(
                out=qts[b][c][:, s0 * QS : s1 * QS],
                in_=qT[b, c * P : (c + 1) * P, s0 * QS : s1 * QS],
            )

        def v_chunk_dma(eng, b, t0, t1):
            eng.dma_start(
                out=vts[b][:, t0:t1, :],
                in_=vv[b, :, t0 * D : t1 * D],
            )

        def k_chunk_dma(eng, b, c, t0, t1):
            eng.dma_start(
                out=kts[b][c][:, t0 * P : t1 * P],
                in_=kk[b, c * P : (c + 1) * P, t0 * P : t1 * P],
            )

        # ---- ALL input DMAs posted up-front, priority-ordered; each
        # queue drains FIFO so order encodes the deadline schedule.
        # Measured (v2): the two HWDGE queues run ~symmetric ~115 GB/s
        # each while draining 1-1.25KB-row descriptors (SDMA engines
        # round-robin the queues per PACKET, so bytes/packet sets the
        # split — the 2-4KB-row bulk transfers late in each list speed
        # that queue up). No SWDGE: a third queue steals packet slots
        # from the critical fill (v1 measured the ACT queue at ~60GB/s
        # with gpsimd's 1MB queued). Deadlines (first MM ~11.4):
        #   k c0+q s0 by 11.4, bias 12.4, v t0-1 by 12.8, v t2-4 by
        #   14.7, k c1 by 16.9, v t5+ by 17.5, q s1 by 21.5, rest 28+.
        # SP queue:
        k_chunk_dma(nc.sync, 0, 0, *kparts[0])          # k0 c0 h0
        q_span_dma(nc.sync, 0, 0, 1, 0)                 # q0 s0 h0
        # q0 s0 h1 is split by column across the queues ~60/40: the SP
        # queue measures ~135 GB/s vs ACT's ~75-90, so an even half split
        # of the critical set leaves the first matmul gated on ACT.
        nc.sync.dma_start(
            out=qts[0][1][:, 0:QS // 2],
            in_=qT[0, P : 2 * P, 0:QS // 2],
        )
        v_chunk_dma(nc.sync, 0, 0, min(2, nkt))         # v0 t0-1 (first O tiles)
        if nkt > 2:
            v_chunk_dma(nc.sync, 0, 2, min(5, nkt))     # v0 t2-4
        if len(kparts) > 1:
            k_chunk_dma(nc.sync, 0, 0, *kparts[1])      # k0 c1 h0
        if nkt > 5:
            v_chunk_dma(nc.sync, 0, 5, nkt)             # v0 t5+
        q_span_dma(nc.sync, 0, 1, 2, 0)                 # q0 s1 h0
        q_span_dma(nc.sync, 0, 1, 2, 1)                 # q0 s1 h1
        for b in range(1, NB):                          # batch-1 k/v
            k_chunk_dma(nc.sync, b, 0, 0, nkt)
            k_chunk_dma(nc.sync, b, 1, 0, nkt)
            v_chunk_dma(nc.sync, b, 0, nkt)
        # ACT queue (the ACT engine must be free for exps by ~11.5us and
        # each post costs it ~0.7us, so only 4 posts go before the
        # compute loop; the bulk-q posts are emitted inside stripe (0,0)
        # between exps, by which point their FIFO position is harmless
        # and their 3-4KB rows drain fast).
        k_chunk_dma(nc.scalar, 0, 1, *kparts[0])        # k0 c0 h1
        nc.scalar.dma_start(
            out=qts[0][1][:, QS // 2 : QS],
            in_=qT[0, P : 2 * P, QS // 2 : QS],
        )                                               # q0 s0 h1 (tail cols)
        nc.scalar.dma_start(out=bias_all, in_=bb[:, :])  # bias
        if len(kparts) > 1:
            k_chunk_dma(nc.scalar, 0, 1, *kparts[1])    # k0 c1 h1

        # PE warmup: dep-free matmuls during the initial DMA fill so the HAM
        # clock gate ramps before the real matmuls start.
        for w in range(NWARM):
            wp = ps_s.tile([P, P], F32, tag="s", name=f"warm{w}")
            nc.tensor.matmul(wp, lhsT=sixteens, rhs=sixteens, start=True, stop=True)

        # ---- compute, one 512-wide q-stripe at a time
        for b in range(NB):
            for s in range(NQS):
                last_stripe = b == NB - 1 and s == NQS - 1
                qoff, qw = s * QS, QS
                qsl = slice(qoff, qoff + qw)
                op0 = ps_o.tile([P, QS], F32, tag="o0", name="op0")[:, :qw]
                op1 = ps_o.tile([P, QS], F32, tag="o1", name="op1")[:, :qw]
                zp = ps_z.tile([P, QS], F32, tag="z", name="zp")[:, :qw]
                acc_e = None

                def s_exp(t):
                    """Emit the S matmul pair + exp for k-tile t; return e."""
                    ksl = slice(t * P, (t + 1) * P)
                    sp = ps_s.tile([P, QS], F32, tag="s", name="sp")[:, :qw]
                    nc.tensor.matmul(
                        sp, lhsT=kts[b][0][:, ksl], rhs=qts[b][0][:, qsl],
                        start=True, stop=False,
                    )
                    nc.tensor.matmul(
                        sp, lhsT=kts[b][1][:, ksl], rhs=qts[b][1][:, qsl],
                        start=False, stop=True,
                    )
                    e = epool.tile([P, QS], BF16, tag="e", name="e")[:, :qw]
                    nc.scalar.activation(e, sp, EXP, bias=biast[b][:, t : t + 1])
                    return e

                # Software-pipelined by FOUR tiles: O(t) is emitted after
                # S(t+4). Two tiles (~1.7us) covers the ~0.9us S->exp
                # latency; the extra depth lets the PE scoreboard keep
                # running S matmuls ahead while early v chunks are still in
                # flight. ps_s bufs=5 holds sp(t)..sp(t+4).
                DEPTH = 4
                pipe = [s_exp(tt) for tt in range(min(DEPTH, nkt))]
                # deferred bulk-q posts on the ACT queue: emitted between
                # exps so the ACT engine's ~0.7us/post cost lands in its
                # per-stripe slack; by now their FIFO position is behind
                # all critical transfers and their 3-4KB rows drain fast.
                if b == 0 and s == 0:
                    q_span_dma(nc.scalar, 0, 2, NQS, 0)   # q0 s2-3 h0
                    q_span_dma(nc.scalar, 0, 2, NQS, 1)   # q0 s2-3 h1
                if b == 0 and s == 1:
                    for b2 in range(1, NB):
                        q_span_dma(nc.scalar, b2, 0, NQS, 0)  # q1 h0 full
                        q_span_dma(nc.scalar, b2, 0, NQS, 1)  # q1 h1 full
                for t in range(nkt):
                    e = pipe.pop(0)
                    if t + DEPTH < nkt:
                        pipe.append(s_exp(t + DEPTH))
                    first, last = t == 0, t == nkt - 1
                    # Z: a running DVE accumulator sums ALL the stripe's
                    # e-tiles so only ONE Z matmul runs, and on the final
                    # tile that Z is emitted BEFORE the O pair (its DVE add
                    # chain resolved ~2 tiles ago thanks to the pipeline):
                    # the normalize reciprocal then overlaps the stripe's
                    # last O matmuls, so the whole recip+mult chain fits
                    # inside the next stripe's 8-matmul prologue window and
                    # the single-buffered O banks are free in time.
                    if acc_e is None:
                        acc_e = e
                    else:
                        na = epool.tile([P, QS], BF16, tag="ep", name="na")[:, :qw]
                        nc.vector.tensor_tensor(na, acc_e, e, ADD)
                        acc_e = na
                    if last:
                        nc.tensor.matmul(
                            zp, lhsT=sixteens, rhs=acc_e, start=True, stop=True,
                        )
                        acc_e = None
                    nc.tensor.matmul(
                        op0, lhsT=vts[b][:, t, 0:P], rhs=e, start=first, stop=last,
                    )
                    nc.tensor.matmul(
                        op1, lhsT=vts[b][:, t, P : 2 * P], rhs=e, start=first, stop=last,
                    )
                # normalize: out = out_unnorm * (1/(16Z)); zp already holds
                # 16Z in every partition. ~18-bit reciprocal, 5x faster than
                # exact; z is far from denorm/inf so approx edge cases can't
                # hit. Processed in chunks so the tail (recip -> mult -> DMA)
                # pipelines; the final stripe uses finer chunks to shorten
                # the drain.
                zbs = zpool.tile([P, QS], F32, tag="zbs", name="zbs")[:, :qw]
                o0 = opool.tile([P, QS], BF16, tag="so0", name="o0")[:, :qw]
                o1 = opool.tile([P, QS], BF16, tag="so1", name="o1")[:, :qw]
                if last_stripe:
                    # Drain: the recip runs concurrently with the final O
                    # matmuls (Z was issued before them); ACT then copies
                    # op1 out of PSUM while the DVE multiplies o0 straight
                    # from PSUM, and o1's multiply runs from SBUF. One DVE
                    # op per output — PSUM-touching DVE ops are latency-
                    # bound at ~0.42us regardless of width, so fewer is
                    # faster (measured: 4 chunked mults cost 1.66us serial).
                    o1s = opool.tile([P, QS], F32, tag="so1c", name="o1s")[:, :qw]
                    COPY = mybir.ActivationFunctionType.Copy
                    nc.vector.reciprocal_approx_fast(out=zbs, in_=zp)
                    nc.scalar.activation(o1s, op1, COPY)
                    nc.vector.tensor_tensor(o0, op0, zbs, MULT)
                    nc.sync.dma_start(out=out[b, s, 0], in_=o0)
                    nc.vector.tensor_tensor(o1, o1s, zbs, MULT)
                    # final output split across both queues: halves move in
                    # parallel, the binding receipt fires ~0.5us earlier
                    nc.scalar.dma_start(
                        out=out[b, s, 1, :, 0 : QS // 2], in_=o1[:, 0 : QS // 2]
                    )
                    nc.sync.dma_start(
                        out=out[b, s, 1, :, QS // 2 : QS], in_=o1[:, QS // 2 : QS]
                    )
                else:
                    nc.vector.reciprocal_approx_fast(out=zbs, in_=zp)
                    nc.vector.tensor_tensor(o0, op0, zbs, MULT)
                    nc.sync.dma_start(out=out[b, s, 0], in_=o0)
                    nc.vector.tensor_tensor(o1, op1, zbs, MULT)
                    nc.scalar.dma_start(out=out[b, s, 1], in_=o1)

    return nc


def build_balanced(slot_nkts, bufmap, nbuf):
    """Stripe-balanced program: 8 independent 512-wide q-stripe SLOTS per
    core, slot j contracting over slot_nkts[j] k-tiles of kv-buffer
    bufmap[j]. With the measured mask (10 batches at 9 k-tiles, 6 at 8),
    slots [9,9,9,9,9,8,8,8] give every core 69 k-tile-units instead of the
    batch-pair scheme's 72 (the 10 nine-batches pigeonhole at least one
    core into 9+9): ~2.7us less PE time on the slowest core.
    """
    from contextlib import ExitStack

    nmax = max(slot_nkts)
    SK = nmax * P
    bufnkt = [max(slot_nkts[j] for j in range(8) if bufmap[j] == bf) for bf in range(nbuf)]
    nc = bacc.Bacc()
    # q in three DRAM pieces: slots 0 and 1 as [P, 1024] (the critical
    # slot-0 column slices then read DRAM at 2KB stride — a 16KB-row
    # packing measurably halved the early fill rate), and slots 2-7 as one
    # [P, 6144] block whose single full-row transfer is fully contiguous.
    q0d = nc.declare_dram_parameter("q0", [P, 2 * QS], BF16, isOutput=False)
    q1d = nc.declare_dram_parameter("q1", [P, 2 * QS], BF16, isOutput=False)
    qrd = nc.declare_dram_parameter("qrest", [P, 6 * 2 * QS], BF16, isOutput=False)
    kk = nc.declare_dram_parameter("k", [nbuf, D, SK], BF16, isOutput=False)
    vv = nc.declare_dram_parameter("v", [nbuf, P, nmax * D], BF16, isOutput=False)
    bb = nc.declare_dram_parameter("bias", [P, nbuf * 32], F32, isOutput=False)
    out = nc.declare_dram_parameter("out", [8, 2, P, QS], BF16, isOutput=True)

    def chunks(lo, hi, n):
        step = max(1, (hi - lo + n - 1) // n)
        return [(a, min(a + step, hi)) for a in range(lo, hi, step)]

    with tile.TileContext(nc) as tc, ExitStack() as ctx:
        consts = ctx.enter_context(tc.tile_pool(name="consts", bufs=1))
        inp = ctx.enter_context(tc.tile_pool(name="inp", bufs=1))
        epool = ctx.enter_context(tc.tile_pool(name="e", bufs=8))
        # o bufs=3: the o0 output DMAs ride the SP queue BEHIND the bulk
        # input bytes and can land ~15us after their stripe; slot j+3's
        # buffer-reuse dependency gives them that slack
        opool = ctx.enter_context(tc.tile_pool(name="o", bufs=3))
        zpool = ctx.enter_context(tc.tile_pool(name="z", bufs=2))
        ps_s = ctx.enter_context(tc.tile_pool(name="ps_s", bufs=5, space="PSUM"))
        ps_o = ctx.enter_context(tc.tile_pool(name="ps_o", bufs=1, space="PSUM"))
        ps_z = ctx.enter_context(tc.tile_pool(name="ps_z", bufs=1, space="PSUM"))

        sixteens = consts.tile([P, P], BF16)
        nc.vector.memset(sixteens, 16.0)
        warm_in = consts.tile([P, 1], F32)
        nc.vector.memset(warm_in, 0.0)
        warm_out = consts.tile([P, 1], F32)
        nc.scalar.activation(warm_out, warm_in, EXP)

        kts = [
            [inp.tile([P, bufnkt[bf] * P], BF16, tag=f"k{bf}{c}", name=f"kt{bf}{c}")
             for c in range(2)]
            for bf in range(nbuf)
        ]
        q0t = inp.tile([P, 2 * QS], BF16, tag="q0", name="q0t")
        q1t = inp.tile([P, 2 * QS], BF16, tag="q1", name="q1t")
        qrt = inp.tile([P, 6 * 2 * QS], BF16, tag="qr", name="qrt")
        vts = [
            inp.tile([P, bufnkt[bf], D], BF16, tag=f"v{bf}", name=f"vt{bf}")
            for bf in range(nbuf)
        ]
        bias_all = inp.tile([P, nbuf * 32], F32, tag="bias", name="bias_all")
        biast = [bias_all[:, bf * 32 : bf * 32 + bufnkt[bf]] for bf in range(nbuf)]

        def q0_dma(eng, c0, c1):
            eng.dma_start(out=q0t[:, c0:c1], in_=q0d[:, c0:c1])

        def v_dma(eng, bf, t0, t1):
            eng.dma_start(out=vts[bf][:, t0:t1, :], in_=vv[bf, :, t0 * D : t1 * D])

        def k_dma(eng, bf, c, t0, t1):
            eng.dma_start(
                out=kts[bf][c][:, t0 * P : t1 * P],
                in_=kk[bf, c * P : (c + 1) * P, t0 * P : t1 * P],
            )

        nk0 = bufnkt[bufmap[0]]
        kparts = chunks(0, nk0, 2)
        # SP queue (priority FIFO; see build() for the rate model)
        k_dma(nc.sync, 0, 0, *kparts[0])                # k b0 c0 h0
        q0_dma(nc.sync, 0, QS)                          # q s0 h0
        q0_dma(nc.sync, QS, QS + QS // 2)               # q s0 h1 head cols
        v_dma(nc.sync, 0, 0, min(2, nk0))               # v b0 t0-1
        if nk0 > 2:
            v_dma(nc.sync, 0, 2, min(5, nk0))           # v b0 t2-4
        if len(kparts) > 1:
            k_dma(nc.sync, 0, 0, *kparts[1])            # k b0 c1 h0
        if nk0 > 5:
            v_dma(nc.sync, 0, 5, nk0)                   # v b0 t5+
        nc.sync.dma_start(out=q1t, in_=q1d[:, :])       # q slot 1
        nc.sync.dma_start(out=qrt, in_=qrd[:, :])       # q slots 2-7, one post
        for bf in range(1, nbuf):                       # remaining kv buffers
            k_dma(nc.sync, bf, 0, 0, bufnkt[bf])
            k_dma(nc.sync, bf, 1, 0, bufnkt[bf])
            v_dma(nc.sync, bf, 0, bufnkt[bf])
        # ACT queue: ONLY these four posts run on the ACT engine — every
        # extra post there delays exps by ~0.6us (measured: two deferred
        # posts cost a 2.8us exp wait / ~1.1us PE stall)
        k_dma(nc.scalar, 0, 1, *kparts[0])              # k b0 c0 h1
        q0_dma(nc.scalar, QS + QS // 2, 2 * QS)         # q s0 h1 tail cols
        nc.scalar.dma_start(out=bias_all, in_=bb[:, :])  # bias
        if len(kparts) > 1:
            k_dma(nc.scalar, 0, 1, *kparts[1])          # k b0 c1 h1

        for w in range(NWARM):
            wp = ps_s.tile([P, P], F32, tag="s", name=f"warm{w}")
            nc.tensor.matmul(wp, lhsT=sixteens, rhs=sixteens, start=True, stop=True)

        for j in range(8):
            nkt = slot_nkts[j]
            bf = bufmap[j]
            last_slot = j == 7
            op0 = ps_o.tile([P, QS], F32, tag="o0", name="op0")
            op1 = ps_o.tile([P, QS], F32, tag="o1", name="op1")
            zp = ps_z.tile([P, QS], F32, tag="z", name="zp")
            acc_e = None

            if j == 0:
                qh0, qh1 = q0t[:, 0:QS], q0t[:, QS : 2 * QS]
            elif j == 1:
                qh0, qh1 = q1t[:, 0:QS], q1t[:, QS : 2 * QS]
            else:
                jo = (j - 2) * 2 * QS
                qh0, qh1 = qrt[:, jo : jo + QS], qrt[:, jo + QS : jo + 2 * QS]

            def s_exp(t):
                ksl = slice(t * P, (t + 1) * P)
                sp = ps_s.tile([P, QS], F32, tag="s", name="sp")
                nc.tensor.matmul(
                    sp, lhsT=kts[bf][0][:, ksl], rhs=qh0,
                    start=True, stop=False,
                )
                nc.tensor.matmul(
                    sp, lhsT=kts[bf][1][:, ksl], rhs=qh1,
                    start=False, stop=True,
                )
                e = epool.tile([P, QS], BF16, tag="e", name="e")
                nc.scalar.activation(e, sp, EXP, bias=biast[bf][:, t : t + 1])
                return e

            DEPTH = 4
            pipe = [s_exp(tt) for tt in range(min(DEPTH, nkt))]
            for t in range(nkt):
                e = pipe.pop(0)
                if t + DEPTH < nkt:
                    pipe.append(s_exp(t + DEPTH))
                first, last = t == 0, t == nkt - 1
                if acc_e is None:
                    acc_e = e
                else:
                    na = epool.tile([P, QS], BF16, tag="ep", name="na")
                    nc.vector.tensor_tensor(na, acc_e, e, ADD)
                    acc_e = na
                if last:
                    nc.tensor.matmul(zp, lhsT=sixteens, rhs=acc_e, start=True, stop=True)
                    acc_e = None
                nc.tensor.matmul(
                    op0, lhsT=vts[bf][:, t, 0:P], rhs=e, start=first, stop=last,
                )
                nc.tensor.matmul(
                    op1, lhsT=vts[bf][:, t, P : 2 * P], rhs=e, start=first, stop=last,
                )
            zbs = zpool.tile([P, QS], F32, tag="zbs", name="zbs")
            o0 = opool.tile([P, QS], BF16, tag="so0", name="o0")
            o1 = opool.tile([P, QS], BF16, tag="so1", name="o1")
            if last_slot:
                o1s = opool.tile([P, QS], F32, tag="so1c", name="o1s")
                COPY = mybir.ActivationFunctionType.Copy
                nc.vector.reciprocal_approx_fast(out=zbs, in_=zp)
                nc.scalar.activation(o1s, op1, COPY)
                nc.vector.tensor_tensor(o0, op0, zbs, MULT)
                nc.sync.dma_start(out=out[j, 0], in_=o0)
                nc.vector.tensor_tensor(o1, o1s, zbs, MULT)
                nc.scalar.dma_start(
                    out=out[j, 1, :, 0 : QS // 2], in_=o1[:, 0 : QS // 2]
                )
                nc.sync.dma_start(
                    out=out[j, 1, :, QS // 2 : QS], in_=o1[:, QS // 2 : QS]
                )
            else:
                nc.vector.reciprocal_approx_fast(out=zbs, in_=zp)
                nc.vector.tensor_tensor(o0, op0, zbs, MULT)
                nc.sync.dma_start(out=out[j, 0], in_=o0)
                nc.vector.tensor_tensor(o1, op1, zbs, MULT)
                nc.scalar.dma_start(out=out[j, 1], in_=o1)

    return nc


def make_in_maps_balanced(q, k, v, mask):
    """Stripe-balanced sharding (see build_balanced). Returns None when the
    mask's per-batch k-tile counts don't fit the {9-tiles: 10, 8-tiles: 6}
    pattern this packing is built for."""
    q = np.asarray(q, dtype=np.float32)
    k = np.asarray(k, dtype=np.float32)
    v = np.asarray(v, dtype=np.float32)
    mask = np.asarray(mask, dtype=np.int32).reshape(len(q), -1)
    B = len(q)
    if B != 16:
        return None
    idxs = [np.nonzero(mask[b])[0] for b in range(B)]
    nktb = [max(1, (len(ix) + P - 1) // P) for ix in idxs]
    order = sorted(range(B), key=lambda b: -len(idxs[b]))
    nine = [b for b in order if nktb[b] == 9]
    eight = [b for b in order if nktb[b] == 8]
    if len(nine) != 10 or len(eight) != 6:
        return None
    slot_nkts = [9, 9, 9, 9, 9, 8, 8, 8]
    bufmap = [0, 0, 0, 0, 1, 2, 2, 3]
    nbuf = 4
    nmax = 9
    SK = nmax * P

    # per-batch compacted k / v / bias, padded to 9 tiles
    kg = np.zeros((B, D, SK), dtype=np.float32)
    vgt = np.zeros((B, P, nmax * D), dtype=np.float32)
    bg = np.full((B, SK), -1.0e9, dtype=np.float32)
    for b in range(B):
        ix = idxs[b]
        kg[b, :, : len(ix)] = k[b][:, ix]
        vg = np.zeros((SK, D), dtype=np.float32)
        vg[: len(ix)] = v[b][ix]
        vgt[b] = vg.reshape(nmax, P, D).transpose(1, 0, 2).reshape(P, -1)
        bg[b, : len(ix)] = -SHIFT
    bgt = bg.reshape(B, nmax, P).transpose(0, 2, 1)  # [B, P, 9]
    qT = np.transpose(q, (0, 2, 1))  # [B, D, S]

    in_maps = []
    slot_maps = []
    for i in range(NCORES):
        bufs = [nine[i], nine[8 + i // 4], eight[i // 2], eight[4 + i // 4]]
        slots = [
            (bufs[0], 0), (bufs[0], 1), (bufs[0], 2), (bufs[0], 3),
            (bufs[1], i % 4),
            (bufs[2], 2 * (i % 2)), (bufs[2], 2 * (i % 2) + 1),
            (bufs[3], i % 4),
        ]
        # slot-minor q pack [P, 8*1024]: slots 2-7 then move as ONE
        # contiguous 12KB-row transfer; slots 0/1 are separate params so
        # their critical column-slices read DRAM at 2KB stride
        qsl = np.zeros((P, 8 * 2 * QS), dtype=np.float32)
        for j, (b, s) in enumerate(slots):
            sl = slice(s * QS, (s + 1) * QS)
            qsl[:, j * 2 * QS : j * 2 * QS + QS] = qT[b, 0:P, sl]
            qsl[:, j * 2 * QS + QS : (j + 1) * 2 * QS] = qT[b, P : 2 * P, sl]
        bias = np.full((P, nbuf * 32), -1.0e9, dtype=np.float32)
        for bf in range(nbuf):
            bias[:, bf * 32 : bf * 32 + nmax] = bgt[bufs[bf]]
        qb = qsl.astype(BF16NP)
        in_maps.append(
            {
                "q0": np.ascontiguousarray(qb[:, 0 : 2 * QS]),
                "q1": np.ascontiguousarray(qb[:, 2 * QS : 4 * QS]),
                "qrest": np.ascontiguousarray(qb[:, 4 * QS :]),
                "k": np.ascontiguousarray(kg[bufs].astype(BF16NP)),
                "v": np.ascontiguousarray(vgt[bufs].astype(BF16NP)),
                "bias": np.ascontiguousarray(bias),
            }
        )
        slot_maps.append(slots)
    return in_maps, slot_nkts, bufmap, nbuf, slot_maps


def make_in_maps(q, k, v, mask):
    """Shard over batch; transpose q; compact the key dim to unmasked keys."""
    q = np.asarray(q, dtype=np.float32)
    k = np.asarray(k, dtype=np.float32)
    v = np.asarray(v, dtype=np.float32)
    mask = np.asarray(mask, dtype=np.int32).reshape(len(q), -1)

    B = len(q)
    idxs = [np.nonzero(mask[b])[0] for b in range(B)]
    n_eff = max((len(ix) for ix in idxs), default=1)
    sk = max(P, ((n_eff + P - 1) // P) * P)  # padded compacted key length

    kg = np.zeros((B, D, sk), dtype=np.float32)
    vg = np.zeros((B, sk, D), dtype=np.float32)
    # exp bias: -SHIFT for real keys, -1e9 for padding (kills it exactly),
    # laid out [P, sk//P] partition-major to match the k-tile slicing
    bg = np.full((B, sk), -1.0e9, dtype=np.float32)
    for b in range(B):
        ix = idxs[b]
        kg[b, :, : len(ix)] = k[b][:, ix]
        vg[b, : len(ix)] = v[b][ix]
        bg[b, : len(ix)] = -SHIFT
    bgt = bg.reshape(B, sk // P, P).transpose(0, 2, 1)  # [B, P, nkt]
    # v partition-major: vgt[b, p, t*D+d] = vg[b, t*128+p, d]
    vgt = vg.reshape(B, sk // P, P, D).transpose(0, 2, 1, 3).reshape(B, P, -1)
    bgp = np.zeros((B, P, P), dtype=np.float32)  # rows padded to 512B lines
    bgp[:, :, : sk // P] = bgt
    # pack per-core as [P, NB*128]: core i gets batches i*NB..i*NB+NB-1
    bgq = bgp.transpose(1, 0, 2).reshape(P, B * P)

    in_maps = []
    for i in range(NCORES):
        sl = slice(i * NB, (i + 1) * NB)
        in_maps.append(
            {
                "qT": np.ascontiguousarray(
                    np.transpose(q[sl], (0, 2, 1)).astype(BF16NP)
                ),
                "k": np.ascontiguousarray(kg[sl].astype(BF16NP)),
                "v": np.ascontiguousarray(vgt[sl].astype(BF16NP)),
                "bias": np.ascontiguousarray(
                    bgq[:, i * NB * P : (i + 1) * NB * P]
                ),
            }
        )
    return in_maps, sk


def run(q, k, v, mask, **kwargs):
    bal = make_in_maps_balanced(q, k, v, mask)
    if bal is not None:
        in_maps, slot_nkts, bufmap, nbuf, slot_maps = bal
        nc = build_balanced(slot_nkts, bufmap, nbuf)
        nc.finalize()
        res = run_bass_kernel_spmd(nc, in_maps, list(range(NCORES)), **kwargs)
        B = len(slot_maps) * 0 + 16
        out = np.zeros((B, D, S), dtype=np.float32)
        for i, r in enumerate(res.results):
            blk = np.asarray(r["out"], dtype=np.float32)  # [8, 2, P, QS]
            for j, (b, s) in enumerate(slot_maps[i]):
                sl = slice(s * QS, (s + 1) * QS)
                out[b, 0:P, sl] = blk[j, 0]
                out[b, P : 2 * P, sl] = blk[j, 1]
        return out, res

    in_maps, sk = make_in_maps(q, k, v, mask)
    nc = build(sk)
    nc.finalize()  # run the Bacc pass pipeline (reg alloc, wait splitting)
    res = run_bass_kernel_spmd(nc, in_maps, list(range(NCORES)), **kwargs)
    # device layout [NB, NQS, 2, P, QS] -> [NB, D, S]
    out = np.concatenate(
        [
            r["out"].transpose(0, 2, 3, 1, 4).reshape(NB, D, S)
            for r in res.results
        ],
        axis=0,
    ).astype(np.float32)
    return out, res


def kernel(q, k, v, mask):
    out, _ = run(q, k, v, mask)
    return out


# revision 37
# speedup vs baseline: 1.0216x; 1.0083x over previous
"""Masked-softmax attention on 8 trn2 NeuronCores.

Reference computation (per batch b):
    att = q @ k                        # [n_q, n_k], k given pre-transposed [d, n_k]
    att = where(mask==0, -1e9, att)
    att = softmax(att, -1) / sqrt(d)
    out = (att @ v).T                  # returned [n_dv, n_q]

Sharding: data-parallel over batch: B=16 -> 2 batches per core x 8 cores.

Host-side, per batch, the key dimension is COMPACTED: masked-out keys
contribute exactly 0 to both the softmax numerator and denominator (the
reference's exp(-1e9 - anything) underflows to +0.0 in fp32), so we gather
only the unmasked columns of k / rows of v, padded up to a multiple of 128
(padding killed by the same -1e9 bias). With a Bernoulli(0.5) mask this
halves the contraction length. Exact, not an approximation.

Device-side plan (per batch). All matmul OPERANDS are bf16 (accumulation
stays fp32 in PSUM): on TRN2's PE both bf16 and f32r run 1 cycle/row at
512-wide moving, but bf16 halves every SBUF fetch and all input DMA, which
removes the SBUF-port contention between the PE's moving-operand stream
and the DVE's e-accumulation traffic. bf16 rounding lands ~1e-2 relative
on the output, inside the 2e-2 gate.

    - Work in the TRANSPOSED score layout S^T[k, q] (k on partitions):
        S^T tile [128k, 512q] = k_slice[d,128k]^T @ qT[d, 512q]  (2 d-chunk accum)
      `k` input [d, n_k] is directly the stationary operand; `q` is transposed
      host-side during sharding so qT[d, n_q] is directly the moving operand.
    - softmax is shift-invariant, so instead of the row max we subtract a
      CONSTANT shift (scores ~ N(0, d) with d=256 -> |s| < ~110 always;
      exp(s-shift) can't overflow and dominant terms can't underflow).
      Mask + shift fold into the scalar-engine exp as a per-partition bias:
        e[k, q] = exp(s + bias_k),  bias_k = -shift - 1e9*(1-mask_k)
    - out^T[dv, q] += v_tile[128k, dv_chunk]^T @ e   (v is directly stationary)
      z[dv, q]    += sixteens[128k, 128]^T @ e       (= 16Z in EVERY partition:
      the all-16s stationary matrix computes the row sum AND broadcasts it,
      folding in the post-softmax 1/sqrt(d)=1/16 scale)
    - out = out^T * (1/z) (DVE approx reciprocal) -> [dv, n_q], the required
      output layout.

DMA schedule (trace-derived, v2): ONLY the two HWDGE queues are used, and
every input DMA is posted up-front in strict priority order — the per-queue
FIFO then drains bytes in deadline order with no gating tricks and no
SWDGE (gpsimd) traffic competing for SDMA engines during the critical
fill. Profiling the v1 kernel showed the first real matmul waiting until
~13.8us because the k chunks were queued behind v and competing with a
1MB gpsimd queue; with the priority-FIFO fill the critical 576KB
(q stripe 0 + k chunk 0, split across both queues) lands ~9.5-10us.
  SP  queue: q0s0h0, k0c0h0, k0c1h0, v0c0, v0c1, q0s1-3h0, q0s1-3h1,
             q1h0, q1h1, k1h0, k1h1, v1  — then the per-stripe o0 output
             DMAs (each waits on its stripe's normalize, so they follow).
  ACT queue: q0s0h1, k0c0h1, bias, k0c1h1 — then per-stripe o1 outputs
             interleaved between exps.
v is pre-transposed host-side to partition-major [P, nkt*D] so its
transfers are fully contiguous. Outputs (bf16) go to a blocked DRAM
layout [NB, NQS, 2, P, QS] so each is one contiguous 1KB-row transfer;
the host reassembles + casts to f32.

gpsimd is NOT used at all: its tensor ops forced a Pool ucode library
swap (UNLOAD_LIB + 27KB LOAD_LIB) right in the final-stripe drain path
(~1.2us) plus a reload in the epilogue.

The inner loop is software-pipelined by FOUR k-tiles (O(t) emitted after
S(t+4), ps_s bufs=5 / ps_o bufs=1 / ps_z 1 = 8 PSUM banks): the in-order
PE queue then never reaches an O matmul before its ~0.9us S->exp chain
resolves. PE warmup matmuls (NWARM) bridge the preamble->first-input
window so the HAM clock gate ramps once and stays at K=8/8.
"""

import numpy as np
import ml_dtypes

import concourse.bacc as bacc
import concourse.mybir as mybir
import concourse.tile as tile
from concourse.bass_utils import run_bass_kernel_spmd

P = 128          # partitions
D = 256          # d == n_dv
S = 2048         # n_q
NB = 2           # batches per core
QS = 512         # q-stripe width (max matmul moving dim into one PSUM bank)
NQS = S // QS    # 4 q-stripes
NCORES = 8
SHIFT = 60.0     # constant softmax shift (see module docstring)
NWARM = 38       # PE warmup matmuls: bridge preamble (~7.6us) -> first
                 # input ready (~11.1us). Must leave NO idle gap before the
                 # real matmuls: the HAM clock gate only reaches K=8/8
                 # after ~3.4us of SUSTAINED PE activity, and a ~1.7us hole
                 # (measured, v2) kept it at half clock until 14.9us.

F32 = mybir.dt.float32
BF16 = mybir.dt.bfloat16
EXP = mybir.ActivationFunctionType.Exp
MULT = mybir.AluOpType.mult
ADD = mybir.AluOpType.add

BF16NP = ml_dtypes.bfloat16


def build(sk):
    """Build the per-core program. sk = compacted key length (mult of 128)."""
    from contextlib import ExitStack

    nkt = sk // P  # number of k-tiles
    nc = bacc.Bacc()
    qT = nc.declare_dram_parameter("qT", [NB, D, S], BF16, isOutput=False)
    kk = nc.declare_dram_parameter("k", [NB, D, sk], BF16, isOutput=False)
    # v pre-transposed host-side to partition-major [P, nkt*D]: v DMAs
    # become fully contiguous on both sides (4.6KB rows, ~2x throughput)
    vv = nc.declare_dram_parameter("v", [NB, P, (sk // P) * D], BF16, isOutput=False)
    # bias pre-packed host-side as [P, NB*128] so one 1KB-line transfer
    # moves both batches' bias columns
    bb = nc.declare_dram_parameter("bias", [P, NB * P], F32, isOutput=False)
    # blocked output layout: one contiguous [P, QS] block per
    # (batch, stripe, dv-half) so each output DMA moves 2KB rows; the host
    # reassembles [NB, D, S] with a cheap numpy transpose.
    out = nc.declare_dram_parameter("out", [NB, NQS, 2, P, QS], BF16, isOutput=True)

    def chunks(lo, hi, n):
        """Split [lo,hi) into n roughly-equal spans (empty spans dropped)."""
        step = max(1, (hi - lo + n - 1) // n)
        return [(a, min(a + step, hi)) for a in range(lo, hi, step)]

    with tile.TileContext(nc) as tc, ExitStack() as ctx:
        consts = ctx.enter_context(tc.tile_pool(name="consts", bufs=1))
        inp = ctx.enter_context(tc.tile_pool(name="inp", bufs=1))
        epool = ctx.enter_context(tc.tile_pool(name="e", bufs=8))
        opool = ctx.enter_context(tc.tile_pool(name="o", bufs=2))
        zpool = ctx.enter_context(tc.tile_pool(name="z", bufs=2))
        # 5 S banks + 2 O banks + 1 Z = 8. Single-buffered O is safe: the
        # normalize DVE ops of stripe s are emitted before stripe s+1's adds
        # on the in-order DVE, so the banks are free ~2.6us before stripe
        # s+1's first O matmul needs them.
        ps_s = ctx.enter_context(tc.tile_pool(name="ps_s", bufs=5, space="PSUM"))
        ps_o = ctx.enter_context(tc.tile_pool(name="ps_o", bufs=1, space="PSUM"))
        ps_z = ctx.enter_context(tc.tile_pool(name="ps_z", bufs=1, space="PSUM"))

        # memset packs the constant into the tile dtype's bits, so bf16
        # is written directly — no f32 staging, sixteens ready ~0.2us
        # after the DVE exits the entry barrier.
        sixteens = consts.tile([P, P], BF16)
        nc.vector.memset(sixteens, 16.0)

        # Warmup Exp: walrus attaches the implicit ACT table load to the
        # first Exp, which eats its sync-wait slots; give it a dep-free one
        # (also hides the ~1.3us table load under the input DMA fill).
        warm_in = consts.tile([P, 1], F32)
        nc.vector.memset(warm_in, 0.0)
        warm_out = consts.tile([P, 1], F32)
        nc.scalar.activation(warm_out, warm_in, EXP)

        # ---- all input tiles, both batches resident simultaneously
        kts = [
            [inp.tile([P, sk], BF16, tag=f"k{b}{c}", name=f"kt{b}{c}") for c in range(2)]
            for b in range(NB)
        ]
        qts = [
            [inp.tile([P, S], BF16, tag=f"q{b}{c}", name=f"qt{b}{c}") for c in range(2)]
            for b in range(NB)
        ]
        vts = [inp.tile([P, nkt, D], BF16, tag=f"v{b}", name=f"vt{b}") for b in range(NB)]
        bias_all = inp.tile([P, NB * P], F32, tag="bias", name="bias_all")
        biast = [bias_all[:, b * P : b * P + nkt] for b in range(NB)]

        kparts = chunks(0, nkt, 2)

        def q_span_dma(eng, b, s0, s1, c):
            eng.dma_start(
                out=qts[b][c][:, s0 * QS : s1 * QS],
                in_=qT[b, c * P : (c + 1) * P, s0 * QS : s1 * QS],
            )

        def v_chunk_dma(eng, b, t0, t1):
            eng.dma_start(
                out=vts[b][:, t0:t1, :],
                in_=vv[b, :, t0 * D : t1 * D],
            )

        def k_chunk_dma(eng, b, c, t0, t1):
            eng.dma_start(
                out=kts[b][c][:, t0 * P : t1 * P],
                in_=kk[b, c * P : (c + 1) * P, t0 * P : t1 * P],
            )

        # ---- ALL input DMAs posted up-front, priority-ordered; each
        # queue drains FIFO so order encodes the deadline schedule.
        # Measured (v2): the two HWDGE queues run ~symmetric ~115 GB/s
        # each while draining 1-1.25KB-row descriptors (SDMA engines
        # round-robin the queues per PACKET, so bytes/packet sets the
        # split — the 2-4KB-row bulk transfers late in each list speed
        # that queue up). No SWDGE: a third queue steals packet slots
        # from the critical fill (v1 measured the ACT queue at ~60GB/s
        # with gpsimd's 1MB queued). Deadlines (first MM ~11.4):
        #   k c0+q s0 by 11.4, bias 12.4, v t0-1 by 12.8, v t2-4 by
        #   14.7, k c1 by 16.9, v t5+ by 17.5, q s1 by 21.5, rest 28+.
        # SP queue:
        k_chunk_dma(nc.sync, 0, 0, *kparts[0])          # k0 c0 h0
        q_span_dma(nc.sync, 0, 0, 1, 0)                 # q0 s0 h0
        # q0 s0 h1 is split by column across the queues ~60/40: the SP
        # queue measures ~135 GB/s vs ACT's ~75-90, so an even half split
        # of the critical set leaves the first matmul gated on ACT.
        nc.sync.dma_start(
            out=qts[0][1][:, 0:QS // 2],
            in_=qT[0, P : 2 * P, 0:QS // 2],
        )
        v_chunk_dma(nc.sync, 0, 0, min(2, nkt))         # v0 t0-1 (first O tiles)
        if nkt > 2:
            v_chunk_dma(nc.sync, 0, 2, min(5, nkt))     # v0 t2-4
        if len(kparts) > 1:
            k_chunk_dma(nc.sync, 0, 0, *kparts[1])      # k0 c1 h0
        if nkt > 5:
            v_chunk_dma(nc.sync, 0, 5, nkt)             # v0 t5+
        q_span_dma(nc.sync, 0, 1, 2, 0)                 # q0 s1 h0
        q_span_dma(nc.sync, 0, 1, 2, 1)                 # q0 s1 h1
        for b in range(1, NB):                          # batch-1 k/v
            k_chunk_dma(nc.sync, b, 0, 0, nkt)
            k_chunk_dma(nc.sync, b, 1, 0, nkt)
            v_chunk_dma(nc.sync, b, 0, nkt)
        # ACT queue (the ACT engine must be free for exps by ~11.5us and
        # each post costs it ~0.7us, so only 4 posts go before the
        # compute loop; the bulk-q posts are emitted inside stripe (0,0)
        # between exps, by which point their FIFO position is harmless
        # and their 3-4KB rows drain fast).
        k_chunk_dma(nc.scalar, 0, 1, *kparts[0])        # k0 c0 h1
        nc.scalar.dma_start(
            out=qts[0][1][:, QS // 2 : QS],
            in_=qT[0, P : 2 * P, QS // 2 : QS],
        )                                               # q0 s0 h1 (tail cols)
        nc.scalar.dma_start(out=bias_all, in_=bb[:, :])  # bias
        if len(kparts) > 1:
            k_chunk_dma(nc.scalar, 0, 1, *kparts[1])    # k0 c1 h1

        # PE warmup: dep-free matmuls during the initial DMA fill so the HAM
        # clock gate ramps before the real matmuls start.
        for w in range(NWARM):
            wp = ps_s.tile([P, P], F32, tag="s", name=f"warm{w}")
            nc.tensor.matmul(wp, lhsT=sixteens, rhs=sixteens, start=True, stop=True)

        # ---- compute, one 512-wide q-stripe at a time
        for b in range(NB):
            for s in range(NQS):
                last_stripe = b == NB - 1 and s == NQS - 1
                qoff, qw = s * QS, QS
                qsl = slice(qoff, qoff + qw)
                op0 = ps_o.tile([P, QS], F32, tag="o0", name="op0")[:, :qw]
                op1 = ps_o.tile([P, QS], F32, tag="o1", name="op1")[:, :qw]
                zp = ps_z.tile([P, QS], F32, tag="z", name="zp")[:, :qw]
                acc_e = None

                def s_exp(t):
                    """Emit the S matmul pair + exp for k-tile t; return e."""
                    ksl = slice(t * P, (t + 1) * P)
                    sp = ps_s.tile([P, QS], F32, tag="s", name="sp")[:, :qw]
                    nc.tensor.matmul(
                        sp, lhsT=kts[b][0][:, ksl], rhs=qts[b][0][:, qsl],
                        start=True, stop=False,
                    )
                    nc.tensor.matmul(
                        sp, lhsT=kts[b][1][:, ksl], rhs=qts[b][1][:, qsl],
                        start=False, stop=True,
                    )
                    e = epool.tile([P, QS], BF16, tag="e", name="e")[:, :qw]
                    nc.scalar.activation(e, sp, EXP, bias=biast[b][:, t : t + 1])
                    return e

                # Software-pipelined by FOUR tiles: O(t) is emitted after
                # S(t+4). Two tiles (~1.7us) covers the ~0.9us S->exp
                # latency; the extra depth lets the PE scoreboard keep
                # running S matmuls ahead while early v chunks are still in
                # flight. ps_s bufs=5 holds sp(t)..sp(t+4).
                DEPTH = 4
                pipe = [s_exp(tt) for tt in range(min(DEPTH, nkt))]
                # deferred bulk-q posts on the ACT queue: emitted between
                # exps so the ACT engine's ~0.7us/post cost lands in its
                # per-stripe slack; by now their FIFO position is behind
                # all critical transfers and their 3-4KB rows drain fast.
                if b == 0 and s == 0:
                    q_span_dma(nc.scalar, 0, 2, NQS, 0)   # q0 s2-3 h0
                    q_span_dma(nc.scalar, 0, 2, NQS, 1)   # q0 s2-3 h1
                if b == 0 and s == 1:
                    for b2 in range(1, NB):
                        q_span_dma(nc.scalar, b2, 0, NQS, 0)  # q1 h0 full
                        q_span_dma(nc.scalar, b2, 0, NQS, 1)  # q1 h1 full
                for t in range(nkt):
                    e = pipe.pop(0)
                    if t + DEPTH < nkt:
                        pipe.append(s_exp(t + DEPTH))
                    first, last = t == 0, t == nkt - 1
                    # Z: a running DVE accumulator sums ALL the stripe's
                    # e-tiles so only ONE Z matmul runs, and on the final
                    # tile that Z is emitted BEFORE the O pair (its DVE add
                    # chain resolved ~2 tiles ago thanks to the pipeline):
                    # the normalize reciprocal then overlaps the stripe's
                    # last O matmuls, so the whole recip+mult chain fits
                    # inside the next stripe's 8-matmul prologue window and
                    # the single-buffered O banks are free in time.
                    if acc_e is None:
                        acc_e = e
                    else:
                        na = epool.tile([P, QS], BF16, tag="ep", name="na")[:, :qw]
                        nc.vector.tensor_tensor(na, acc_e, e, ADD)
                        acc_e = na
                    if last:
                        nc.tensor.matmul(
                            zp, lhsT=sixteens, rhs=acc_e, start=True, stop=True,
                        )
                        acc_e = None
                    nc.tensor.matmul(
                        op0, lhsT=vts[b][:, t, 0:P], rhs=e, start=first, stop=last,
                    )
                    nc.tensor.matmul(
                        op1, lhsT=vts[b][:, t, P : 2 * P], rhs=e, start=first, stop=last,
                    )
                # normalize: out = out_unnorm * (1/(16Z)); zp already holds
                # 16Z in every partition. ~18-bit reciprocal, 5x faster than
                # exact; z is far from denorm/inf so approx edge cases can't
                # hit. Processed in chunks so the tail (recip -> mult -> DMA)
                # pipelines; the final stripe uses finer chunks to shorten
                # the drain.
                zbs = zpool.tile([P, QS], F32, tag="zbs", name="zbs")[:, :qw]
                o0 = opool.tile([P, QS], BF16, tag="so0", name="o0")[:, :qw]
                o1 = opool.tile([P, QS], BF16, tag="so1", name="o1")[:, :qw]
                if last_stripe:
                    # Drain: the recip runs concurrently with the final O
                    # matmuls (Z was issued before them); ACT then copies
                    # op1 out of PSUM while the DVE multiplies o0 straight
                    # from PSUM, and o1's multiply runs from SBUF. One DVE
                    # op per output — PSUM-touching DVE ops are latency-
                    # bound at ~0.42us regardless of width, so fewer is
                    # faster (measured: 4 chunked mults cost 1.66us serial).
                    o1s = opool.tile([P, QS], F32, tag="so1c", name="o1s")[:, :qw]
                    COPY = mybir.ActivationFunctionType.Copy
                    nc.vector.reciprocal_approx_fast(out=zbs, in_=zp)
                    nc.scalar.activation(o1s, op1, COPY)
                    nc.vector.tensor_tensor(o0, op0, zbs, MULT)
                    nc.sync.dma_start(out=out[b, s, 0], in_=o0)
                    nc.vector.tensor_tensor(o1, o1s, zbs, MULT)
                    # final output split across both queues: halves move in
                    # parallel, the binding receipt fires ~0.5us earlier
                    nc.scalar.dma_start(
                        out=out[b, s, 1, :, 0 : QS // 2], in_=o1[:, 0 : QS // 2]
                    )
                    nc.sync.dma_start(
                        out=out[b, s, 1, :, QS // 2 : QS], in_=o1[:, QS // 2 : QS]
                    )
                else:
                    nc.vector.reciprocal_approx_fast(out=zbs, in_=zp)
                    nc.vector.tensor_tensor(o0, op0, zbs, MULT)
                    nc.sync.dma_start(out=out[b, s, 0], in_=o0)
                    nc.vector.tensor_tensor(o1, op1, zbs, MULT)
                    nc.scalar.dma_start(out=out[b, s, 1], in_=o1)

    return nc


def build_balanced(slot_nkts, bufmap, nbuf):
    """Stripe-balanced program: 8 independent 512-wide q-stripe SLOTS per
    core, slot j contracting over slot_nkts[j] k-tiles of kv-buffer
    bufmap[j]. With the measured mask (10 batches at 9 k-tiles, 6 at 8),
    slots [9,9,9,9,9,8,8,8] give every core 69 k-tile-units instead of the
    batch-pair scheme's 72 (the 10 nine-batches pigeonhole at least one
    core into 9+9): ~2.7us less PE time on the slowest core.
    """
    from contextlib import ExitStack

    nmax = max(slot_nkts)
    SK = nmax * P
    bufnkt = [max(slot_nkts[j] for j in range(8) if bufmap[j] == bf) for bf in range(nbuf)]
    nc = bacc.Bacc()
    # q in three DRAM pieces: slots 0 and 1 as [P, 1024] (the critical
    # slot-0 column slices then read DRAM at 2KB stride — a 16KB-row
    # packing measurably halved the early fill rate), and slots 2-7 as one
    # [P, 6144] block whose single full-row transfer is fully contiguous.
    # every critical-fill transfer is its own DENSE param (contiguous DRAM
    # reads keep HBM row-buffer hits high; strided 1-2KB segment reads
    # measurably run the queues at half rate)
    q0a = nc.declare_dram_parameter("q0a", [P, QS], BF16, isOutput=False)
    q0b = nc.declare_dram_parameter("q0b", [P, QS // 2], BF16, isOutput=False)
    q0c = nc.declare_dram_parameter("q0c", [P, QS // 2], BF16, isOutput=False)
    q1d = nc.declare_dram_parameter("q1", [P, 2 * QS], BF16, isOutput=False)
    qrd = nc.declare_dram_parameter("qrest", [P, 6 * 2 * QS], BF16, isOutput=False)
    k0p = [
        nc.declare_dram_parameter(f"k{c}{h}", [P, w], BF16, isOutput=False)
        for c, h, w in (
            (0, 0, 5 * P), (0, 1, 4 * P), (1, 0, 5 * P), (1, 1, 4 * P),
        )
    ]  # buf0 k chunk pieces: (half, chunk) -> [P, chunk_cols]
    v0p = [
        nc.declare_dram_parameter(f"v0{i}", [P, w], BF16, isOutput=False)
        for i, w in ((0, 2 * D), (1, 3 * D), (2, 4 * D))
    ]  # buf0 v pieces: t0-1, t2-4, t5-8
    kk = nc.declare_dram_parameter("k", [nbuf, D, SK], BF16, isOutput=False)
    vv = nc.declare_dram_parameter("v", [nbuf, P, nmax * D], BF16, isOutput=False)
    bb = nc.declare_dram_parameter("bias", [P, nbuf * 32], F32, isOutput=False)
    out = nc.declare_dram_parameter("out", [8, 2, P, QS], BF16, isOutput=True)

    def chunks(lo, hi, n):
        step = max(1, (hi - lo + n - 1) // n)
        return [(a, min(a + step, hi)) for a in range(lo, hi, step)]

    with tile.TileContext(nc) as tc, ExitStack() as ctx:
        consts = ctx.enter_context(tc.tile_pool(name="consts", bufs=1))
        inp = ctx.enter_context(tc.tile_pool(name="inp", bufs=1))
        epool = ctx.enter_context(tc.tile_pool(name="e", bufs=8))
        # o bufs=3: the o0 output DMAs ride the SP queue BEHIND the bulk
        # input bytes and can land ~15us after their stripe; slot j+3's
        # buffer-reuse dependency gives them that slack
        opool = ctx.enter_context(tc.tile_pool(name="o", bufs=3))
        zpool = ctx.enter_context(tc.tile_pool(name="z", bufs=2))
        ps_s = ctx.enter_context(tc.tile_pool(name="ps_s", bufs=5, space="PSUM"))
        ps_o = ctx.enter_context(tc.tile_pool(name="ps_o", bufs=1, space="PSUM"))
        ps_z = ctx.enter_context(tc.tile_pool(name="ps_z", bufs=1, space="PSUM"))

        sixteens = consts.tile([P, P], BF16)
        nc.vector.memset(sixteens, 16.0)
        warm_in = consts.tile([P, 1], F32)
        nc.vector.memset(warm_in, 0.0)
        warm_out = consts.tile([P, 1], F32)
        nc.scalar.activation(warm_out, warm_in, EXP)

        kts = [
            [inp.tile([P, nmax * P], BF16, tag=f"k{bf}{c}", name=f"kt{bf}{c}")
             for c in range(2)]
            for bf in range(nbuf)
        ]
        q0t = inp.tile([P, 2 * QS], BF16, tag="q0", name="q0t")
        q1t = inp.tile([P, 2 * QS], BF16, tag="q1", name="q1t")
        qrt = inp.tile([P, 6 * 2 * QS], BF16, tag="qr", name="qrt")
        vts = [
            inp.tile([P, nmax, D], BF16, tag=f"v{bf}", name=f"vt{bf}")
            for bf in range(nbuf)
        ]
        bias_all = inp.tile([P, nbuf * 32], F32, tag="bias", name="bias_all")
        biast = [bias_all[:, bf * 32 : bf * 32 + bufnkt[bf]] for bf in range(nbuf)]

        def q0_dma(eng, c0, c1):
            eng.dma_start(out=q0t[:, c0:c1], in_=q0d[:, c0:c1])

        def v_dma(eng, bf, t0, t1):
            eng.dma_start(out=vts[bf][:, t0:t1, :], in_=vv[bf, :, t0 * D : t1 * D])

        def k_dma(eng, bf, c, t0, t1):
            eng.dma_start(
                out=kts[bf][c][:, t0 * P : t1 * P],
                in_=kk[bf, c * P : (c + 1) * P, t0 * P : t1 * P],
            )

        nk0 = bufnkt[bufmap[0]]
        assert nk0 == 9 and nmax == 9
        # SP queue (priority FIFO; dense-param critical pieces)
        nc.sync.dma_start(out=kts[0][0][:, 0 : 5 * P], in_=k0p[0][:, :])  # k0 c0 h0
        nc.sync.dma_start(out=q0t[:, 0:QS], in_=q0a[:, :])                # q s0 h0
        nc.sync.dma_start(out=q0t[:, QS : QS + QS // 2], in_=q0b[:, :])   # q s0 h1a
        nc.sync.dma_start(out=vts[0][:, 0:2, :], in_=v0p[0][:, :])        # v0 t0-1
        nc.sync.dma_start(out=vts[0][:, 2:5, :], in_=v0p[1][:, :])        # v0 t2-4
        nc.sync.dma_start(out=kts[0][0][:, 5 * P : 9 * P], in_=k0p[1][:, :])  # k0 c1 h0
        nc.sync.dma_start(out=vts[0][:, 5:9, :], in_=v0p[2][:, :])        # v0 t5-8
        nc.sync.dma_start(out=q1t, in_=q1d[:, :])       # q slot 1
        nc.sync.dma_start(out=qrt, in_=qrd[:, :])       # q slots 2-7, one post
        for bf in range(1, nbuf):                       # remaining kv buffers
            # transfer the full SK columns even when the slot only uses 8
            # tiles: full rows = dense DRAM reads
            k_dma(nc.sync, bf, 0, 0, nmax)
            k_dma(nc.sync, bf, 1, 0, nmax)
            v_dma(nc.sync, bf, 0, nmax)
        # ACT queue: ONLY these four posts run on the ACT engine — every
        # extra post there delays exps by ~0.6us (measured: two deferred
        # posts cost a 2.8us exp wait / ~1.1us PE stall)
        nc.scalar.dma_start(out=kts[0][1][:, 0 : 5 * P], in_=k0p[2][:, :])  # k0 c0 h1
        nc.scalar.dma_start(out=q0t[:, QS + QS // 2 : 2 * QS], in_=q0c[:, :])  # q s0 h1b
        nc.scalar.dma_start(out=bias_all, in_=bb[:, :])  # bias
        nc.scalar.dma_start(out=kts[0][1][:, 5 * P : 9 * P], in_=k0p[3][:, :])  # k0 c1 h1

        for w in range(NWARM):
            wp = ps_s.tile([P, P], F32, tag="s", name=f"warm{w}")
            nc.tensor.matmul(wp, lhsT=sixteens, rhs=sixteens, start=True, stop=True)

        for j in range(8):
            nkt = slot_nkts[j]
            bf = bufmap[j]
            last_slot = j == 7
            op0 = ps_o.tile([P, QS], F32, tag="o0", name="op0")
            op1 = ps_o.tile([P, QS], F32, tag="o1", name="op1")
            zp = ps_z.tile([P, QS], F32, tag="z", name="zp")
            acc_e = None

            if j == 0:
                qh0, qh1 = q0t[:, 0:QS], q0t[:, QS : 2 * QS]
            elif j == 1:
                qh0, qh1 = q1t[:, 0:QS], q1t[:, QS : 2 * QS]
            else:
                jo = (j - 2) * 2 * QS
                qh0, qh1 = qrt[:, jo : jo + QS], qrt[:, jo + QS : jo + 2 * QS]

            def s_exp(t):
                ksl = slice(t * P, (t + 1) * P)
                sp = ps_s.tile([P, QS], F32, tag="s", name="sp")
                nc.tensor.matmul(
                    sp, lhsT=kts[bf][0][:, ksl], rhs=qh0,
                    start=True, stop=False,
                )
                nc.tensor.matmul(
                    sp, lhsT=kts[bf][1][:, ksl], rhs=qh1,
                    start=False, stop=True,
                )
                e = epool.tile([P, QS], BF16, tag="e", name="e")
                nc.scalar.activation(e, sp, EXP, bias=biast[bf][:, t : t + 1])
                return e

            DEPTH = 4
            pipe = [s_exp(tt) for tt in range(min(DEPTH, nkt))]
            for t in range(nkt):
                e = pipe.pop(0)
                if t + DEPTH < nkt:
                    pipe.append(s_exp(t + DEPTH))
                first, last = t == 0, t == nkt - 1
                if acc_e is None:
                    acc_e = e
                else:
                    na = epool.tile([P, QS], BF16, tag="ep", name="na")
                    nc.vector.tensor_tensor(na, acc_e, e, ADD)
                    acc_e = na
                if last:
                    nc.tensor.matmul(zp, lhsT=sixteens, rhs=acc_e, start=True, stop=True)
                    acc_e = None
                nc.tensor.matmul(
                    op0, lhsT=vts[bf][:, t, 0:P], rhs=e, start=first, stop=last,
                )
                nc.tensor.matmul(
                    op1, lhsT=vts[bf][:, t, P : 2 * P], rhs=e, start=first, stop=last,
                )
            zbs = zpool.tile([P, QS], F32, tag="zbs", name="zbs")
            o0 = opool.tile([P, QS], BF16, tag="so0", name="o0")
            o1 = opool.tile([P, QS], BF16, tag="so1", name="o1")
            if last_slot:
                o1s = opool.tile([P, QS], F32, tag="so1c", name="o1s")
                COPY = mybir.ActivationFunctionType.Copy
                nc.vector.reciprocal_approx_fast(out=zbs, in_=zp)
                nc.scalar.activation(o1s, op1, COPY)
                nc.vector.tensor_tensor(o0, op0, zbs, MULT)
                nc.sync.dma_start(out=out[j, 0], in_=o0)
                nc.vector.tensor_tensor(o1, o1s, zbs, MULT)
                nc.scalar.dma_start(
                    out=out[j, 1, :, 0 : QS // 2], in_=o1[:, 0 : QS // 2]
                )
                nc.sync.dma_start(
                    out=out[j, 1, :, QS // 2 : QS], in_=o1[:, QS // 2 : QS]
                )
            else:
                nc.vector.reciprocal_approx_fast(out=zbs, in_=zp)
                nc.vector.tensor_tensor(o0, op0, zbs, MULT)
                nc.sync.dma_start(out=out[j, 0], in_=o0)
                nc.vector.tensor_tensor(o1, op1, zbs, MULT)
                nc.scalar.dma_start(out=out[j, 1], in_=o1)

    return nc


def make_in_maps_balanced(q, k, v, mask):
    """Stripe-balanced sharding (see build_balanced). Returns None when the
    mask's per-batch k-tile counts don't fit the {9-tiles: 10, 8-tiles: 6}
    pattern this packing is built for."""
    q = np.asarray(q, dtype=np.float32)
    k = np.asarray(k, dtype=np.float32)
    v = np.asarray(v, dtype=np.float32)
    mask = np.asarray(mask, dtype=np.int32).reshape(len(q), -1)
    B = len(q)
    if B != 16:
        return None
    idxs = [np.nonzero(mask[b])[0] for b in range(B)]
    nktb = [max(1, (len(ix) + P - 1) // P) for ix in idxs]
    order = sorted(range(B), key=lambda b: -len(idxs[b]))
    nine = [b for b in order if nktb[b] == 9]
    eight = [b for b in order if nktb[b] == 8]
    if len(nine) != 10 or len(eight) != 6:
        return None
    slot_nkts = [9, 9, 9, 9, 9, 8, 8, 8]
    bufmap = [0, 0, 0, 0, 1, 2, 2, 3]
    nbuf = 4
    nmax = 9
    SK = nmax * P

    # per-batch compacted k / v / bias, padded to 9 tiles
    kg = np.zeros((B, D, SK), dtype=np.float32)
    vgt = np.zeros((B, P, nmax * D), dtype=np.float32)
    bg = np.full((B, SK), -1.0e9, dtype=np.float32)
    for b in range(B):
        ix = idxs[b]
        kg[b, :, : len(ix)] = k[b][:, ix]
        vg = np.zeros((SK, D), dtype=np.float32)
        vg[: len(ix)] = v[b][ix]
        vgt[b] = vg.reshape(nmax, P, D).transpose(1, 0, 2).reshape(P, -1)
        bg[b, : len(ix)] = -SHIFT
    bgt = bg.reshape(B, nmax, P).transpose(0, 2, 1)  # [B, P, 9]
    qT = np.transpose(q, (0, 2, 1))  # [B, D, S]

    in_maps = []
    slot_maps = []
    for i in range(NCORES):
        bufs = [nine[i], nine[8 + i // 4], eight[i // 2], eight[4 + i // 4]]
        slots = [
            (bufs[0], 0), (bufs[0], 1), (bufs[0], 2), (bufs[0], 3),
            (bufs[1], i % 4),
            (bufs[2], 2 * (i % 2)), (bufs[2], 2 * (i % 2) + 1),
            (bufs[3], i % 4),
        ]
        # slot-minor q pack [P, 8*1024]: slots 2-7 then move as ONE
        # contiguous 12KB-row transfer; slots 0/1 are separate params so
        # their critical column-slices read DRAM at 2KB stride
        qsl = np.zeros((P, 8 * 2 * QS), dtype=np.float32)
        for j, (b, s) in enumerate(slots):
            sl = slice(s * QS, (s + 1) * QS)
            qsl[:, j * 2 * QS : j * 2 * QS + QS] = qT[b, 0:P, sl]
            qsl[:, j * 2 * QS + QS : (j + 1) * 2 * QS] = qT[b, P : 2 * P, sl]
        bias = np.full((P, nbuf * 32), -1.0e9, dtype=np.float32)
        for bf in range(nbuf):
            bias[:, bf * 32 : bf * 32 + nmax] = bgt[bufs[bf]]
        qb = qsl.astype(BF16NP)
        k0 = kg[bufs[0]].astype(BF16NP)   # [D, SK]
        v0 = vgt[bufs[0]].astype(BF16NP)  # [P, 9*D]
        in_maps.append(
            {
                "q0a": np.ascontiguousarray(qb[:, 0:QS]),
                "q0b": np.ascontiguousarray(qb[:, QS : QS + QS // 2]),
                "q0c": np.ascontiguousarray(qb[:, QS + QS // 2 : 2 * QS]),
                "q1": np.ascontiguousarray(qb[:, 2 * QS : 4 * QS]),
                "qrest": np.ascontiguousarray(qb[:, 4 * QS :]),
                "k00": np.ascontiguousarray(k0[0:P, 0 : 5 * P]),
                "k01": np.ascontiguousarray(k0[0:P, 5 * P : 9 * P]),
                "k10": np.ascontiguousarray(k0[P : 2 * P, 0 : 5 * P]),
                "k11": np.ascontiguousarray(k0[P : 2 * P, 5 * P : 9 * P]),
                "v00": np.ascontiguousarray(v0[:, 0 : 2 * D]),
                "v01": np.ascontiguousarray(v0[:, 2 * D : 5 * D]),
                "v02": np.ascontiguousarray(v0[:, 5 * D : 9 * D]),
                "k": np.ascontiguousarray(kg[bufs].astype(BF16NP)),
                "v": np.ascontiguousarray(vgt[bufs].astype(BF16NP)),
                "bias": np.ascontiguousarray(bias),
            }
        )
        slot_maps.append(slots)
    return in_maps, slot_nkts, bufmap, nbuf, slot_maps


def make_in_maps(q, k, v, mask):
    """Shard over batch; transpose q; compact the key dim to unmasked keys."""
    q = np.asarray(q, dtype=np.float32)
    k = np.asarray(k, dtype=np.float32)
    v = np.asarray(v, dtype=np.float32)
    mask = np.asarray(mask, dtype=np.int32).reshape(len(q), -1)

    B = len(q)
    idxs = [np.nonzero(mask[b])[0] for b in range(B)]
    n_eff = max((len(ix) for ix in idxs), default=1)
    sk = max(P, ((n_eff + P - 1) // P) * P)  # padded compacted key length

    kg = np.zeros((B, D, sk), dtype=np.float32)
    vg = np.zeros((B, sk, D), dtype=np.float32)
    # exp bias: -SHIFT for real keys, -1e9 for padding (kills it exactly),
    # laid out [P, sk//P] partition-major to match the k-tile slicing
    bg = np.full((B, sk), -1.0e9, dtype=np.float32)
    for b in range(B):
        ix = idxs[b]
        kg[b, :, : len(ix)] = k[b][:, ix]
        vg[b, : len(ix)] = v[b][ix]
        bg[b, : len(ix)] = -SHIFT
    bgt = bg.reshape(B, sk // P, P).transpose(0, 2, 1)  # [B, P, nkt]
    # v partition-major: vgt[b, p, t*D+d] = vg[b, t*128+p, d]
    vgt = vg.reshape(B, sk // P, P, D).transpose(0, 2, 1, 3).reshape(B, P, -1)
    bgp = np.zeros((B, P, P), dtype=np.float32)  # rows padded to 512B lines
    bgp[:, :, : sk // P] = bgt
    # pack per-core as [P, NB*128]: core i gets batches i*NB..i*NB+NB-1
    bgq = bgp.transpose(1, 0, 2).reshape(P, B * P)

    in_maps = []
    for i in range(NCORES):
        sl = slice(i * NB, (i + 1) * NB)
        in_maps.append(
            {
                "qT": np.ascontiguousarray(
                    np.transpose(q[sl], (0, 2, 1)).astype(BF16NP)
                ),
                "k": np.ascontiguousarray(kg[sl].astype(BF16NP)),
                "v": np.ascontiguousarray(vgt[sl].astype(BF16NP)),
                "bias": np.ascontiguousarray(
                    bgq[:, i * NB * P : (i + 1) * NB * P]
                ),
            }
        )
    return in_maps, sk


def run(q, k, v, mask, **kwargs):
    bal = make_in_maps_balanced(q, k, v, mask)
    if bal is not None:
        in_maps, slot_nkts, bufmap, nbuf, slot_maps = bal
        nc = build_balanced(slot_nkts, bufmap, nbuf)
        nc.finalize()
        res = run_bass_kernel_spmd(nc, in_maps, list(range(NCORES)), **kwargs)
        B = len(slot_maps) * 0 + 16
        out = np.zeros((B, D, S), dtype=np.float32)
        for i, r in enumerate(res.results):
            blk = np.asarray(r["out"], dtype=np.float32)  # [8, 2, P, QS]
            for j, (b, s) in enumerate(slot_maps[i]):
                sl = slice(s * QS, (s + 1) * QS)
                out[b, 0:P, sl] = blk[j, 0]
                out[b, P : 2 * P, sl] = blk[j, 1]
        return out, res

    in_maps, sk = make_in_maps(q, k, v, mask)
    nc = build(sk)
    nc.finalize()  # run the Bacc pass pipeline (reg alloc, wait splitting)
    res = run_bass_kernel_spmd(nc, in_maps, list(range(NCORES)), **kwargs)
    # device layout [NB, NQS, 2, P, QS] -> [NB, D, S]
    out = np.concatenate(
        [
            r["out"].transpose(0, 2, 3, 1, 4).reshape(NB, D, S)
            for r in res.results
        ],
        axis=0,
    ).astype(np.float32)
    return out, res


def kernel(q, k, v, mask):
    out, _ = run(q, k, v, mask)
    return out


# revision 39
# speedup vs baseline: 1.0246x; 1.0030x over previous
"""Masked-softmax attention on 8 trn2 NeuronCores.

Reference computation (per batch b):
    att = q @ k                        # [n_q, n_k], k given pre-transposed [d, n_k]
    att = where(mask==0, -1e9, att)
    att = softmax(att, -1) / sqrt(d)
    out = (att @ v).T                  # returned [n_dv, n_q]

Sharding: data-parallel over batch: B=16 -> 2 batches per core x 8 cores.

Host-side, per batch, the key dimension is COMPACTED: masked-out keys
contribute exactly 0 to both the softmax numerator and denominator (the
reference's exp(-1e9 - anything) underflows to +0.0 in fp32), so we gather
only the unmasked columns of k / rows of v, padded up to a multiple of 128
(padding killed by the same -1e9 bias). With a Bernoulli(0.5) mask this
halves the contraction length. Exact, not an approximation.

Device-side plan (per batch). All matmul OPERANDS are bf16 (accumulation
stays fp32 in PSUM): on TRN2's PE both bf16 and f32r run 1 cycle/row at
512-wide moving, but bf16 halves every SBUF fetch and all input DMA, which
removes the SBUF-port contention between the PE's moving-operand stream
and the DVE's e-accumulation traffic. bf16 rounding lands ~1e-2 relative
on the output, inside the 2e-2 gate.

    - Work in the TRANSPOSED score layout S^T[k, q] (k on partitions):
        S^T tile [128k, 512q] = k_slice[d,128k]^T @ qT[d, 512q]  (2 d-chunk accum)
      `k` input [d, n_k] is directly the stationary operand; `q` is transposed
      host-side during sharding so qT[d, n_q] is directly the moving operand.
    - softmax is shift-invariant, so instead of the row max we subtract a
      CONSTANT shift (scores ~ N(0, d) with d=256 -> |s| < ~110 always;
      exp(s-shift) can't overflow and dominant terms can't underflow).
      Mask + shift fold into the scalar-engine exp as a per-partition bias:
        e[k, q] = exp(s + bias_k),  bias_k = -shift - 1e9*(1-mask_k)
    - out^T[dv, q] += v_tile[128k, dv_chunk]^T @ e   (v is directly stationary)
      z[dv, q]    += sixteens[128k, 128]^T @ e       (= 16Z in EVERY partition:
      the all-16s stationary matrix computes the row sum AND broadcasts it,
      folding in the post-softmax 1/sqrt(d)=1/16 scale)
    - out = out^T * (1/z) (DVE approx reciprocal) -> [dv, n_q], the required
      output layout.

DMA schedule (trace-derived, v2): ONLY the two HWDGE queues are used, and
every input DMA is posted up-front in strict priority order — the per-queue
FIFO then drains bytes in deadline order with no gating tricks and no
SWDGE (gpsimd) traffic competing for SDMA engines during the critical
fill. Profiling the v1 kernel showed the first real matmul waiting until
~13.8us because the k chunks were queued behind v and competing with a
1MB gpsimd queue; with the priority-FIFO fill the critical 576KB
(q stripe 0 + k chunk 0, split across both queues) lands ~9.5-10us.
  SP  queue: q0s0h0, k0c0h0, k0c1h0, v0c0, v0c1, q0s1-3h0, q0s1-3h1,
             q1h0, q1h1, k1h0, k1h1, v1  — then the per-stripe o0 output
             DMAs (each waits on its stripe's normalize, so they follow).
  ACT queue: q0s0h1, k0c0h1, bias, k0c1h1 — then per-stripe o1 outputs
             interleaved between exps.
v is pre-transposed host-side to partition-major [P, nkt*D] so its
transfers are fully contiguous. Outputs (bf16) go to a blocked DRAM
layout [NB, NQS, 2, P, QS] so each is one contiguous 1KB-row transfer;
the host reassembles + casts to f32.

gpsimd is NOT used at all: its tensor ops forced a Pool ucode library
swap (UNLOAD_LIB + 27KB LOAD_LIB) right in the final-stripe drain path
(~1.2us) plus a reload in the epilogue.

The inner loop is software-pipelined by FOUR k-tiles (O(t) emitted after
S(t+4), ps_s bufs=5 / ps_o bufs=1 / ps_z 1 = 8 PSUM banks): the in-order
PE queue then never reaches an O matmul before its ~0.9us S->exp chain
resolves. PE warmup matmuls (NWARM) bridge the preamble->first-input
window so the HAM clock gate ramps once and stays at K=8/8.
"""

import numpy as np
import ml_dtypes

import concourse.bacc as bacc
import concourse.mybir as mybir
import concourse.tile as tile
from concourse.bass_utils import run_bass_kernel_spmd

P = 128          # partitions
D = 256          # d == n_dv
S = 2048         # n_q
NB = 2           # batches per core
QS = 512         # q-stripe width (max matmul moving dim into one PSUM bank)
NQS = S // QS    # 4 q-stripes
NCORES = 8
SHIFT = 60.0     # constant softmax shift (see module docstring)
NWARM = 44       # PE warmup matmuls: bridge preamble (~7.6us) -> first
                 # input ready (~11.8-12.0us). Must leave NO idle gap
                 # before the real matmuls: the HAM clock gate reaches
                 # K=8/8 only after ~3.4us of SUSTAINED PE activity, and
                 # any pre-warm hole defers it (measured: a 0.8us hole
                 # once pushed warm-up to 16.5us, ~4.5us of half-clock
                 # matmuls). 44 cold warmups span to ~11.6us worst case;
                 # once warm fires they shrink to ~53ns so the overshoot
                 # self-corrects.

F32 = mybir.dt.float32
BF16 = mybir.dt.bfloat16
EXP = mybir.ActivationFunctionType.Exp
MULT = mybir.AluOpType.mult
ADD = mybir.AluOpType.add

BF16NP = ml_dtypes.bfloat16


def build(sk):
    """Build the per-core program. sk = compacted key length (mult of 128)."""
    from contextlib import ExitStack

    nkt = sk // P  # number of k-tiles
    nc = bacc.Bacc()
    qT = nc.declare_dram_parameter("qT", [NB, D, S], BF16, isOutput=False)
    kk = nc.declare_dram_parameter("k", [NB, D, sk], BF16, isOutput=False)
    # v pre-transposed host-side to partition-major [P, nkt*D]: v DMAs
    # become fully contiguous on both sides (4.6KB rows, ~2x throughput)
    vv = nc.declare_dram_parameter("v", [NB, P, (sk // P) * D], BF16, isOutput=False)
    # bias pre-packed host-side as [P, NB*128] so one 1KB-line transfer
    # moves both batches' bias columns
    bb = nc.declare_dram_parameter("bias", [P, NB * P], F32, isOutput=False)
    # blocked output layout: one contiguous [P, QS] block per
    # (batch, stripe, dv-half) so each output DMA moves 2KB rows; the host
    # reassembles [NB, D, S] with a cheap numpy transpose.
    out = nc.declare_dram_parameter("out", [NB, NQS, 2, P, QS], BF16, isOutput=True)

    def chunks(lo, hi, n):
        """Split [lo,hi) into n roughly-equal spans (empty spans dropped)."""
        step = max(1, (hi - lo + n - 1) // n)
        return [(a, min(a + step, hi)) for a in range(lo, hi, step)]

    with tile.TileContext(nc) as tc, ExitStack() as ctx:
        consts = ctx.enter_context(tc.tile_pool(name="consts", bufs=1))
        inp = ctx.enter_context(tc.tile_pool(name="inp", bufs=1))
        epool = ctx.enter_context(tc.tile_pool(name="e", bufs=8))
        opool = ctx.enter_context(tc.tile_pool(name="o", bufs=2))
        zpool = ctx.enter_context(tc.tile_pool(name="z", bufs=2))
        # 5 S banks + 2 O banks + 1 Z = 8. Single-buffered O is safe: the
        # normalize DVE ops of stripe s are emitted before stripe s+1's adds
        # on the in-order DVE, so the banks are free ~2.6us before stripe
        # s+1's first O matmul needs them.
        ps_s = ctx.enter_context(tc.tile_pool(name="ps_s", bufs=5, space="PSUM"))
        ps_o = ctx.enter_context(tc.tile_pool(name="ps_o", bufs=1, space="PSUM"))
        ps_z = ctx.enter_context(tc.tile_pool(name="ps_z", bufs=1, space="PSUM"))

        # memset packs the constant into the tile dtype's bits, so bf16
        # is written directly — no f32 staging, sixteens ready ~0.2us
        # after the DVE exits the entry barrier.
        sixteens = consts.tile([P, P], BF16)
        nc.vector.memset(sixteens, 16.0)

        # Warmup Exp: walrus attaches the implicit ACT table load to the
        # first Exp, which eats its sync-wait slots; give it a dep-free one
        # (also hides the ~1.3us table load under the input DMA fill).
        warm_in = consts.tile([P, 1], F32)
        nc.vector.memset(warm_in, 0.0)
        warm_out = consts.tile([P, 1], F32)
        nc.scalar.activation(warm_out, warm_in, EXP)

        # ---- all input tiles, both batches resident simultaneously
        kts = [
            [inp.tile([P, sk], BF16, tag=f"k{b}{c}", name=f"kt{b}{c}") for c in range(2)]
            for b in range(NB)
        ]
        qts = [
            [inp.tile([P, S], BF16, tag=f"q{b}{c}", name=f"qt{b}{c}") for c in range(2)]
            for b in range(NB)
        ]
        vts = [inp.tile([P, nkt, D], BF16, tag=f"v{b}", name=f"vt{b}") for b in range(NB)]
        bias_all = inp.tile([P, NB * P], F32, tag="bias", name="bias_all")
        biast = [bias_all[:, b * P : b * P + nkt] for b in range(NB)]

        kparts = chunks(0, nkt, 2)

        def q_span_dma(eng, b, s0, s1, c):
            eng.dma_start(
                out=qts[b][c][:, s0 * QS : s1 * QS],
                in_=qT[b, c * P : (c + 1) * P, s0 * QS : s1 * QS],
            )

        def v_chunk_dma(eng, b, t0, t1):
            eng.dma_start(
                out=vts[b][:, t0:t1, :],
                in_=vv[b, :, t0 * D : t1 * D],
            )

        def k_chunk_dma(eng, b, c, t0, t1):
            eng.dma_start(
                out=kts[b][c][:, t0 * P : t1 * P],
                in_=kk[b, c * P : (c + 1) * P, t0 * P : t1 * P],
            )

        # ---- ALL input DMAs posted up-front, priority-ordered; each
        # queue drains FIFO so order encodes the deadline schedule.
        # Measured (v2): the two HWDGE queues run ~symmetric ~115 GB/s
        # each while draining 1-1.25KB-row descriptors (SDMA engines
        # round-robin the queues per PACKET, so bytes/packet sets the
        # split — the 2-4KB-row bulk transfers late in each list speed
        # that queue up). No SWDGE: a third queue steals packet slots
        # from the critical fill (v1 measured the ACT queue at ~60GB/s
        # with gpsimd's 1MB queued). Deadlines (first MM ~11.4):
        #   k c0+q s0 by 11.4, bias 12.4, v t0-1 by 12.8, v t2-4 by
        #   14.7, k c1 by 16.9, v t5+ by 17.5, q s1 by 21.5, rest 28+.
        # SP queue:
        k_chunk_dma(nc.sync, 0, 0, *kparts[0])          # k0 c0 h0
        q_span_dma(nc.sync, 0, 0, 1, 0)                 # q0 s0 h0
        # q0 s0 h1 is split by column across the queues ~60/40: the SP
        # queue measures ~135 GB/s vs ACT's ~75-90, so an even half split
        # of the critical set leaves the first matmul gated on ACT.
        nc.sync.dma_start(
            out=qts[0][1][:, 0:QS // 2],
            in_=qT[0, P : 2 * P, 0:QS // 2],
        )
        v_chunk_dma(nc.sync, 0, 0, min(2, nkt))         # v0 t0-1 (first O tiles)
        if nkt > 2:
            v_chunk_dma(nc.sync, 0, 2, min(5, nkt))     # v0 t2-4
        if len(kparts) > 1:
            k_chunk_dma(nc.sync, 0, 0, *kparts[1])      # k0 c1 h0
        if nkt > 5:
            v_chunk_dma(nc.sync, 0, 5, nkt)             # v0 t5+
        q_span_dma(nc.sync, 0, 1, 2, 0)                 # q0 s1 h0
        q_span_dma(nc.sync, 0, 1, 2, 1)                 # q0 s1 h1
        for b in range(1, NB):                          # batch-1 k/v
            k_chunk_dma(nc.sync, b, 0, 0, nkt)
            k_chunk_dma(nc.sync, b, 1, 0, nkt)
            v_chunk_dma(nc.sync, b, 0, nkt)
        # ACT queue (the ACT engine must be free for exps by ~11.5us and
        # each post costs it ~0.7us, so only 4 posts go before the
        # compute loop; the bulk-q posts are emitted inside stripe (0,0)
        # between exps, by which point their FIFO position is harmless
        # and their 3-4KB rows drain fast).
        k_chunk_dma(nc.scalar, 0, 1, *kparts[0])        # k0 c0 h1
        nc.scalar.dma_start(
            out=qts[0][1][:, QS // 2 : QS],
            in_=qT[0, P : 2 * P, QS // 2 : QS],
        )                                               # q0 s0 h1 (tail cols)
        nc.scalar.dma_start(out=bias_all, in_=bb[:, :])  # bias
        if len(kparts) > 1:
            k_chunk_dma(nc.scalar, 0, 1, *kparts[1])    # k0 c1 h1

        # PE warmup: dep-free matmuls during the initial DMA fill so the HAM
        # clock gate ramps before the real matmuls start.
        for w in range(NWARM):
            wp = ps_s.tile([P, P], F32, tag="s", name=f"warm{w}")
            nc.tensor.matmul(wp, lhsT=sixteens, rhs=sixteens, start=True, stop=True)

        # ---- compute, one 512-wide q-stripe at a time
        for b in range(NB):
            for s in range(NQS):
                last_stripe = b == NB - 1 and s == NQS - 1
                qoff, qw = s * QS, QS
                qsl = slice(qoff, qoff + qw)
                op0 = ps_o.tile([P, QS], F32, tag="o0", name="op0")[:, :qw]
                op1 = ps_o.tile([P, QS], F32, tag="o1", name="op1")[:, :qw]
                zp = ps_z.tile([P, QS], F32, tag="z", name="zp")[:, :qw]
                acc_e = None

                def s_exp(t):
                    """Emit the S matmul pair + exp for k-tile t; return e."""
                    ksl = slice(t * P, (t + 1) * P)
                    sp = ps_s.tile([P, QS], F32, tag="s", name="sp")[:, :qw]
                    nc.tensor.matmul(
                        sp, lhsT=kts[b][0][:, ksl], rhs=qts[b][0][:, qsl],
                        start=True, stop=False,
                    )
                    nc.tensor.matmul(
                        sp, lhsT=kts[b][1][:, ksl], rhs=qts[b][1][:, qsl],
                        start=False, stop=True,
                    )
                    e = epool.tile([P, QS], BF16, tag="e", name="e")[:, :qw]
                    nc.scalar.activation(e, sp, EXP, bias=biast[b][:, t : t + 1])
                    return e

                # Software-pipelined by FOUR tiles: O(t) is emitted after
                # S(t+4). Two tiles (~1.7us) covers the ~0.9us S->exp
                # latency; the extra depth lets the PE scoreboard keep
                # running S matmuls ahead while early v chunks are still in
                # flight. ps_s bufs=5 holds sp(t)..sp(t+4).
                DEPTH = 4
                pipe = [s_exp(tt) for tt in range(min(DEPTH, nkt))]
                # deferred bulk-q posts on the ACT queue: emitted between
                # exps so the ACT engine's ~0.7us/post cost lands in its
                # per-stripe slack; by now their FIFO position is behind
                # all critical transfers and their 3-4KB rows drain fast.
                if b == 0 and s == 0:
                    q_span_dma(nc.scalar, 0, 2, NQS, 0)   # q0 s2-3 h0
                    q_span_dma(nc.scalar, 0, 2, NQS, 1)   # q0 s2-3 h1
                if b == 0 and s == 1:
                    for b2 in range(1, NB):
                        q_span_dma(nc.scalar, b2, 0, NQS, 0)  # q1 h0 full
                        q_span_dma(nc.scalar, b2, 0, NQS, 1)  # q1 h1 full
                for t in range(nkt):
                    e = pipe.pop(0)
                    if t + DEPTH < nkt:
                        pipe.append(s_exp(t + DEPTH))
                    first, last = t == 0, t == nkt - 1
                    # Z: a running DVE accumulator sums ALL the stripe's
                    # e-tiles so only ONE Z matmul runs, and on the final
                    # tile that Z is emitted BEFORE the O pair (its DVE add
                    # chain resolved ~2 tiles ago thanks to the pipeline):
                    # the normalize reciprocal then overlaps the stripe's
                    # last O matmuls, so the whole recip+mult chain fits
                    # inside the next stripe's 8-matmul prologue window and
                    # the single-buffered O banks are free in time.
                    if acc_e is None:
                        acc_e = e
                    else:
                        na = epool.tile([P, QS], BF16, tag="ep", name="na")[:, :qw]
                        nc.vector.tensor_tensor(na, acc_e, e, ADD)
                        acc_e = na
                    if last:
                        nc.tensor.matmul(
                            zp, lhsT=sixteens, rhs=acc_e, start=True, stop=True,
                        )
                        acc_e = None
                    nc.tensor.matmul(
                        op0, lhsT=vts[b][:, t, 0:P], rhs=e, start=first, stop=last,
                    )
                    nc.tensor.matmul(
                        op1, lhsT=vts[b][:, t, P : 2 * P], rhs=e, start=first, stop=last,
                    )
                # normalize: out = out_unnorm * (1/(16Z)); zp already holds
                # 16Z in every partition. ~18-bit reciprocal, 5x faster than
                # exact; z is far from denorm/inf so approx edge cases can't
                # hit. Processed in chunks so the tail (recip -> mult -> DMA)
                # pipelines; the final stripe uses finer chunks to shorten
                # the drain.
                zbs = zpool.tile([P, QS], F32, tag="zbs", name="zbs")[:, :qw]
                o0 = opool.tile([P, QS], BF16, tag="so0", name="o0")[:, :qw]
                o1 = opool.tile([P, QS], BF16, tag="so1", name="o1")[:, :qw]
                if last_stripe:
                    # Drain: the recip runs concurrently with the final O
                    # matmuls (Z was issued before them); ACT then copies
                    # op1 out of PSUM while the DVE multiplies o0 straight
                    # from PSUM, and o1's multiply runs from SBUF. One DVE
                    # op per output — PSUM-touching DVE ops are latency-
                    # bound at ~0.42us regardless of width, so fewer is
                    # faster (measured: 4 chunked mults cost 1.66us serial).
                    o1s = opool.tile([P, QS], F32, tag="so1c", name="o1s")[:, :qw]
                    COPY = mybir.ActivationFunctionType.Copy
                    nc.vector.reciprocal_approx_fast(out=zbs, in_=zp)
                    nc.scalar.activation(o1s, op1, COPY)
                    nc.vector.tensor_tensor(o0, op0, zbs, MULT)
                    nc.sync.dma_start(out=out[b, s, 0], in_=o0)
                    nc.vector.tensor_tensor(o1, o1s, zbs, MULT)
                    # final output split across both queues: halves move in
                    # parallel, the binding receipt fires ~0.5us earlier
                    nc.scalar.dma_start(
                        out=out[b, s, 1, :, 0 : QS // 2], in_=o1[:, 0 : QS // 2]
                    )
                    nc.sync.dma_start(
                        out=out[b, s, 1, :, QS // 2 : QS], in_=o1[:, QS // 2 : QS]
                    )
                else:
                    nc.vector.reciprocal_approx_fast(out=zbs, in_=zp)
                    nc.vector.tensor_tensor(o0, op0, zbs, MULT)
                    nc.sync.dma_start(out=out[b, s, 0], in_=o0)
                    nc.vector.tensor_tensor(o1, op1, zbs, MULT)
                    nc.scalar.dma_start(out=out[b, s, 1], in_=o1)

    return nc


def build_balanced(slot_nkts, bufmap, nbuf):
    """Stripe-balanced program: 8 independent 512-wide q-stripe SLOTS per
    core, slot j contracting over slot_nkts[j] k-tiles of kv-buffer
    bufmap[j]. With the measured mask (10 batches at 9 k-tiles, 6 at 8),
    slots [9,9,9,9,9,8,8,8] give every core 69 k-tile-units instead of the
    batch-pair scheme's 72 (the 10 nine-batches pigeonhole at least one
    core into 9+9): ~2.7us less PE time on the slowest core.
    """
    from contextlib import ExitStack

    nmax = max(slot_nkts)
    SK = nmax * P
    bufnkt = [max(slot_nkts[j] for j in range(8) if bufmap[j] == bf) for bf in range(nbuf)]
    nc = bacc.Bacc()
    # q in three DRAM pieces: slots 0 and 1 as [P, 1024] (the critical
    # slot-0 column slices then read DRAM at 2KB stride — a 16KB-row
    # packing measurably halved the early fill rate), and slots 2-7 as one
    # [P, 6144] block whose single full-row transfer is fully contiguous.
    # every critical-fill transfer is its own DENSE param (contiguous DRAM
    # reads keep HBM row-buffer hits high; strided 1-2KB segment reads
    # measurably run the queues at half rate)
    q0a = nc.declare_dram_parameter("q0a", [P, QS], BF16, isOutput=False)
    q0b = nc.declare_dram_parameter("q0b", [P, QS // 2], BF16, isOutput=False)
    q0c = nc.declare_dram_parameter("q0c", [P, QS // 2], BF16, isOutput=False)
    q1d = nc.declare_dram_parameter("q1", [P, 2 * QS], BF16, isOutput=False)
    qrd = nc.declare_dram_parameter("qrest", [P, 6 * 2 * QS], BF16, isOutput=False)
    k0p = [
        nc.declare_dram_parameter(f"k{c}{h}", [P, w], BF16, isOutput=False)
        for c, h, w in (
            (0, 0, 5 * P), (0, 1, 4 * P), (1, 0, 5 * P), (1, 1, 4 * P),
        )
    ]  # buf0 k chunk pieces: (half, chunk) -> [P, chunk_cols]
    v0p = [
        nc.declare_dram_parameter(f"v0{i}", [P, w], BF16, isOutput=False)
        for i, w in ((0, 2 * D), (1, 3 * D), (2, 4 * D))
    ]  # buf0 v pieces: t0-1, t2-4, t5-8
    kk = nc.declare_dram_parameter("k", [nbuf, D, SK], BF16, isOutput=False)
    vv = nc.declare_dram_parameter("v", [nbuf, P, nmax * D], BF16, isOutput=False)
    bb = nc.declare_dram_parameter("bias", [P, nbuf * 32], F32, isOutput=False)
    out = nc.declare_dram_parameter("out", [8, 2, P, QS], BF16, isOutput=True)

    def chunks(lo, hi, n):
        step = max(1, (hi - lo + n - 1) // n)
        return [(a, min(a + step, hi)) for a in range(lo, hi, step)]

    with tile.TileContext(nc) as tc, ExitStack() as ctx:
        consts = ctx.enter_context(tc.tile_pool(name="consts", bufs=1))
        inp = ctx.enter_context(tc.tile_pool(name="inp", bufs=1))
        epool = ctx.enter_context(tc.tile_pool(name="e", bufs=8))
        # o bufs=3: the o0 output DMAs ride the SP queue BEHIND the bulk
        # input bytes and can land ~15us after their stripe; slot j+3's
        # buffer-reuse dependency gives them that slack
        opool = ctx.enter_context(tc.tile_pool(name="o", bufs=3))
        zpool = ctx.enter_context(tc.tile_pool(name="z", bufs=2))
        ps_s = ctx.enter_context(tc.tile_pool(name="ps_s", bufs=5, space="PSUM"))
        ps_o = ctx.enter_context(tc.tile_pool(name="ps_o", bufs=1, space="PSUM"))
        ps_z = ctx.enter_context(tc.tile_pool(name="ps_z", bufs=1, space="PSUM"))

        sixteens = consts.tile([P, P], BF16)
        nc.vector.memset(sixteens, 16.0)
        warm_in = consts.tile([P, 1], F32)
        nc.vector.memset(warm_in, 0.0)
        warm_out = consts.tile([P, 1], F32)
        nc.scalar.activation(warm_out, warm_in, EXP)

        kts = [
            [inp.tile([P, nmax * P], BF16, tag=f"k{bf}{c}", name=f"kt{bf}{c}")
             for c in range(2)]
            for bf in range(nbuf)
        ]
        q0t = inp.tile([P, 2 * QS], BF16, tag="q0", name="q0t")
        q1t = inp.tile([P, 2 * QS], BF16, tag="q1", name="q1t")
        qrt = inp.tile([P, 6 * 2 * QS], BF16, tag="qr", name="qrt")
        vts = [
            inp.tile([P, nmax, D], BF16, tag=f"v{bf}", name=f"vt{bf}")
            for bf in range(nbuf)
        ]
        bias_all = inp.tile([P, nbuf * 32], F32, tag="bias", name="bias_all")
        biast = [bias_all[:, bf * 32 : bf * 32 + bufnkt[bf]] for bf in range(nbuf)]

        def v_dma(eng, bf, t0, t1):
            eng.dma_start(out=vts[bf][:, t0:t1, :], in_=vv[bf, :, t0 * D : t1 * D])

        def k_dma(eng, bf, c, t0, t1):
            eng.dma_start(
                out=kts[bf][c][:, t0 * P : t1 * P],
                in_=kk[bf, c * P : (c + 1) * P, t0 * P : t1 * P],
            )

        nk0 = bufnkt[bufmap[0]]
        assert nk0 == 9 and nmax == 9
        # SP queue (priority FIFO; dense-param critical pieces)
        nc.sync.dma_start(out=kts[0][0][:, 0 : 5 * P], in_=k0p[0][:, :])  # k0 c0 h0
        nc.sync.dma_start(out=q0t[:, 0:QS], in_=q0a[:, :])                # q s0 h0
        nc.sync.dma_start(out=q0t[:, QS : QS + QS // 2], in_=q0b[:, :])   # q s0 h1a
        nc.sync.dma_start(out=vts[0][:, 0:2, :], in_=v0p[0][:, :])        # v0 t0-1
        nc.sync.dma_start(out=vts[0][:, 2:5, :], in_=v0p[1][:, :])        # v0 t2-4
        nc.sync.dma_start(out=kts[0][0][:, 5 * P : 9 * P], in_=k0p[1][:, :])  # k0 c1 h0
        nc.sync.dma_start(out=vts[0][:, 5:9, :], in_=v0p[2][:, :])        # v0 t5-8
        nc.sync.dma_start(out=q1t, in_=q1d[:, :])       # q slot 1
        nc.sync.dma_start(out=qrt, in_=qrd[:, :])       # q slots 2-7, one post
        for bf in range(1, nbuf):                       # remaining kv buffers
            # transfer the full SK columns even when the slot only uses 8
            # tiles: full rows = dense DRAM reads
            k_dma(nc.sync, bf, 0, 0, nmax)
            k_dma(nc.sync, bf, 1, 0, nmax)
            v_dma(nc.sync, bf, 0, nmax)
        # ACT queue: ONLY these four posts run on the ACT engine — every
        # extra post there delays exps by ~0.6us (measured: two deferred
        # posts cost a 2.8us exp wait / ~1.1us PE stall)
        nc.scalar.dma_start(out=kts[0][1][:, 0 : 5 * P], in_=k0p[2][:, :])  # k0 c0 h1
        nc.scalar.dma_start(out=q0t[:, QS + QS // 2 : 2 * QS], in_=q0c[:, :])  # q s0 h1b
        nc.scalar.dma_start(out=bias_all, in_=bb[:, :])  # bias
        nc.scalar.dma_start(out=kts[0][1][:, 5 * P : 9 * P], in_=k0p[3][:, :])  # k0 c1 h1

        for w in range(NWARM):
            wp = ps_s.tile([P, P], F32, tag="s", name=f"warm{w}")
            nc.tensor.matmul(wp, lhsT=sixteens, rhs=sixteens, start=True, stop=True)

        for j in range(8):
            nkt = slot_nkts[j]
            bf = bufmap[j]
            last_slot = j == 7
            op0 = ps_o.tile([P, QS], F32, tag="o0", name="op0")
            op1 = ps_o.tile([P, QS], F32, tag="o1", name="op1")
            zp = ps_z.tile([P, QS], F32, tag="z", name="zp")
            acc_e = None

            if j == 0:
                qh0, qh1 = q0t[:, 0:QS], q0t[:, QS : 2 * QS]
            elif j == 1:
                qh0, qh1 = q1t[:, 0:QS], q1t[:, QS : 2 * QS]
            else:
                jo = (j - 2) * 2 * QS
                qh0, qh1 = qrt[:, jo : jo + QS], qrt[:, jo + QS : jo + 2 * QS]

            def s_exp(t):
                ksl = slice(t * P, (t + 1) * P)
                sp = ps_s.tile([P, QS], F32, tag="s", name="sp")
                nc.tensor.matmul(
                    sp, lhsT=kts[bf][0][:, ksl], rhs=qh0,
                    start=True, stop=False,
                )
                nc.tensor.matmul(
                    sp, lhsT=kts[bf][1][:, ksl], rhs=qh1,
                    start=False, stop=True,
                )
                e = epool.tile([P, QS], BF16, tag="e", name="e")
                nc.scalar.activation(e, sp, EXP, bias=biast[bf][:, t : t + 1])
                return e

            DEPTH = 4
            pipe = [s_exp(tt) for tt in range(min(DEPTH, nkt))]
            for t in range(nkt):
                e = pipe.pop(0)
                if t + DEPTH < nkt:
                    pipe.append(s_exp(t + DEPTH))
                first, last = t == 0, t == nkt - 1
                if acc_e is None:
                    acc_e = e
                else:
                    na = epool.tile([P, QS], BF16, tag="ep", name="na")
                    nc.vector.tensor_tensor(na, acc_e, e, ADD)
                    acc_e = na
                if last:
                    nc.tensor.matmul(zp, lhsT=sixteens, rhs=acc_e, start=True, stop=True)
                    acc_e = None
                nc.tensor.matmul(
                    op0, lhsT=vts[bf][:, t, 0:P], rhs=e, start=first, stop=last,
                )
                nc.tensor.matmul(
                    op1, lhsT=vts[bf][:, t, P : 2 * P], rhs=e, start=first, stop=last,
                )
            zbs = zpool.tile([P, QS], F32, tag="zbs", name="zbs")
            o0 = opool.tile([P, QS], BF16, tag="so0", name="o0")
            o1 = opool.tile([P, QS], BF16, tag="so1", name="o1")
            if last_slot:
                o1s = opool.tile([P, QS], F32, tag="so1c", name="o1s")
                COPY = mybir.ActivationFunctionType.Copy
                nc.vector.reciprocal_approx_fast(out=zbs, in_=zp)
                nc.scalar.activation(o1s, op1, COPY)
                nc.vector.tensor_tensor(o0, op0, zbs, MULT)
                nc.sync.dma_start(out=out[j, 0], in_=o0)
                nc.vector.tensor_tensor(o1, o1s, zbs, MULT)
                nc.scalar.dma_start(
                    out=out[j, 1, :, 0 : QS // 2], in_=o1[:, 0 : QS // 2]
                )
                nc.sync.dma_start(
                    out=out[j, 1, :, QS // 2 : QS], in_=o1[:, QS // 2 : QS]
                )
            else:
                nc.vector.reciprocal_approx_fast(out=zbs, in_=zp)
                nc.vector.tensor_tensor(o0, op0, zbs, MULT)
                nc.sync.dma_start(out=out[j, 0], in_=o0)
                nc.vector.tensor_tensor(o1, op1, zbs, MULT)
                nc.scalar.dma_start(out=out[j, 1], in_=o1)

    return nc


def make_in_maps_balanced(q, k, v, mask):
    """Stripe-balanced sharding (see build_balanced). Returns None when the
    mask's per-batch k-tile counts don't fit the {9-tiles: 10, 8-tiles: 6}
    pattern this packing is built for."""
    q = np.asarray(q, dtype=np.float32)
    k = np.asarray(k, dtype=np.float32)
    v = np.asarray(v, dtype=np.float32)
    mask = np.asarray(mask, dtype=np.int32).reshape(len(q), -1)
    B = len(q)
    if B != 16:
        return None
    idxs = [np.nonzero(mask[b])[0] for b in range(B)]
    nktb = [max(1, (len(ix) + P - 1) // P) for ix in idxs]
    order = sorted(range(B), key=lambda b: -len(idxs[b]))
    nine = [b for b in order if nktb[b] == 9]
    eight = [b for b in order if nktb[b] == 8]
    if len(nine) != 10 or len(eight) != 6:
        return None
    slot_nkts = [9, 9, 9, 9, 9, 8, 8, 8]
    bufmap = [0, 0, 0, 0, 1, 2, 2, 3]
    nbuf = 4
    nmax = 9
    SK = nmax * P

    # per-batch compacted k / v / bias, padded to 9 tiles
    kg = np.zeros((B, D, SK), dtype=np.float32)
    vgt = np.zeros((B, P, nmax * D), dtype=np.float32)
    bg = np.full((B, SK), -1.0e9, dtype=np.float32)
    for b in range(B):
        ix = idxs[b]
        kg[b, :, : len(ix)] = k[b][:, ix]
        vg = np.zeros((SK, D), dtype=np.float32)
        vg[: len(ix)] = v[b][ix]
        vgt[b] = vg.reshape(nmax, P, D).transpose(1, 0, 2).reshape(P, -1)
        bg[b, : len(ix)] = -SHIFT
    bgt = bg.reshape(B, nmax, P).transpose(0, 2, 1)  # [B, P, 9]
    qT = np.transpose(q, (0, 2, 1))  # [B, D, S]

    in_maps = []
    slot_maps = []
    for i in range(NCORES):
        bufs = [nine[i], nine[8 + i // 4], eight[i // 2], eight[4 + i // 4]]
        slots = [
            (bufs[0], 0), (bufs[0], 1), (bufs[0], 2), (bufs[0], 3),
            (bufs[1], i % 4),
            (bufs[2], 2 * (i % 2)), (bufs[2], 2 * (i % 2) + 1),
            (bufs[3], i % 4),
        ]
        # slot-minor q pack [P, 8*1024]: slots 2-7 then move as ONE
        # contiguous 12KB-row transfer; slots 0/1 are separate params so
        # their critical column-slices read DRAM at 2KB stride
        qsl = np.zeros((P, 8 * 2 * QS), dtype=np.float32)
        for j, (b, s) in enumerate(slots):
            sl = slice(s * QS, (s + 1) * QS)
            qsl[:, j * 2 * QS : j * 2 * QS + QS] = qT[b, 0:P, sl]
            qsl[:, j * 2 * QS + QS : (j + 1) * 2 * QS] = qT[b, P : 2 * P, sl]
        bias = np.full((P, nbuf * 32), -1.0e9, dtype=np.float32)
        for bf in range(nbuf):
            bias[:, bf * 32 : bf * 32 + nmax] = bgt[bufs[bf]]
        qb = qsl.astype(BF16NP)
        k0 = kg[bufs[0]].astype(BF16NP)   # [D, SK]
        v0 = vgt[bufs[0]].astype(BF16NP)  # [P, 9*D]
        in_maps.append(
            {
                "q0a": np.ascontiguousarray(qb[:, 0:QS]),
                "q0b": np.ascontiguousarray(qb[:, QS : QS + QS // 2]),
                "q0c": np.ascontiguousarray(qb[:, QS + QS // 2 : 2 * QS]),
                "q1": np.ascontiguousarray(qb[:, 2 * QS : 4 * QS]),
                "qrest": np.ascontiguousarray(qb[:, 4 * QS :]),
                "k00": np.ascontiguousarray(k0[0:P, 0 : 5 * P]),
                "k01": np.ascontiguousarray(k0[0:P, 5 * P : 9 * P]),
                "k10": np.ascontiguousarray(k0[P : 2 * P, 0 : 5 * P]),
                "k11": np.ascontiguousarray(k0[P : 2 * P, 5 * P : 9 * P]),
                "v00": np.ascontiguousarray(v0[:, 0 : 2 * D]),
                "v01": np.ascontiguousarray(v0[:, 2 * D : 5 * D]),
                "v02": np.ascontiguousarray(v0[:, 5 * D : 9 * D]),
                "k": np.ascontiguousarray(kg[bufs].astype(BF16NP)),
                "v": np.ascontiguousarray(vgt[bufs].astype(BF16NP)),
                "bias": np.ascontiguousarray(bias),
            }
        )
        slot_maps.append(slots)
    return in_maps, slot_nkts, bufmap, nbuf, slot_maps


def make_in_maps(q, k, v, mask):
    """Shard over batch; transpose q; compact the key dim to unmasked keys."""
    q = np.asarray(q, dtype=np.float32)
    k = np.asarray(k, dtype=np.float32)
    v = np.asarray(v, dtype=np.float32)
    mask = np.asarray(mask, dtype=np.int32).reshape(len(q), -1)

    B = len(q)
    idxs = [np.nonzero(mask[b])[0] for b in range(B)]
    n_eff = max((len(ix) for ix in idxs), default=1)
    sk = max(P, ((n_eff + P - 1) // P) * P)  # padded compacted key length

    kg = np.zeros((B, D, sk), dtype=np.float32)
    vg = np.zeros((B, sk, D), dtype=np.float32)
    # exp bias: -SHIFT for real keys, -1e9 for padding (kills it exactly),
    # laid out [P, sk//P] partition-major to match the k-tile slicing
    bg = np.full((B, sk), -1.0e9, dtype=np.float32)
    for b in range(B):
        ix = idxs[b]
        kg[b, :, : len(ix)] = k[b][:, ix]
        vg[b, : len(ix)] = v[b][ix]
        bg[b, : len(ix)] = -SHIFT
    bgt = bg.reshape(B, sk // P, P).transpose(0, 2, 1)  # [B, P, nkt]
    # v partition-major: vgt[b, p, t*D+d] = vg[b, t*128+p, d]
    vgt = vg.reshape(B, sk // P, P, D).transpose(0, 2, 1, 3).reshape(B, P, -1)
    bgp = np.zeros((B, P, P), dtype=np.float32)  # rows padded to 512B lines
    bgp[:, :, : sk // P] = bgt
    # pack per-core as [P, NB*128]: core i gets batches i*NB..i*NB+NB-1
    bgq = bgp.transpose(1, 0, 2).reshape(P, B * P)

    in_maps = []
    for i in range(NCORES):
        sl = slice(i * NB, (i + 1) * NB)
        in_maps.append(
            {
                "qT": np.ascontiguousarray(
                    np.transpose(q[sl], (0, 2, 1)).astype(BF16NP)
                ),
                "k": np.ascontiguousarray(kg[sl].astype(BF16NP)),
                "v": np.ascontiguousarray(vgt[sl].astype(BF16NP)),
                "bias": np.ascontiguousarray(
                    bgq[:, i * NB * P : (i + 1) * NB * P]
                ),
            }
        )
    return in_maps, sk


def run(q, k, v, mask, **kwargs):
    bal = make_in_maps_balanced(q, k, v, mask)
    if bal is not None:
        in_maps, slot_nkts, bufmap, nbuf, slot_maps = bal
        nc = build_balanced(slot_nkts, bufmap, nbuf)
        nc.finalize()
        res = run_bass_kernel_spmd(nc, in_maps, list(range(NCORES)), **kwargs)
        B = len(slot_maps) * 0 + 16
        out = np.zeros((B, D, S), dtype=np.float32)
        for i, r in enumerate(res.results):
            blk = np.asarray(r["out"], dtype=np.float32)  # [8, 2, P, QS]
            for j, (b, s) in enumerate(slot_maps[i]):
                sl = slice(s * QS, (s + 1) * QS)
                out[b, 0:P, sl] = blk[j, 0]
                out[b, P : 2 * P, sl] = blk[j, 1]
        return out, res

    in_maps, sk = make_in_maps(q, k, v, mask)
    nc = build(sk)
    nc.finalize()  # run the Bacc pass pipeline (reg alloc, wait splitting)
    res = run_bass_kernel_spmd(nc, in_maps, list(range(NCORES)), **kwargs)
    # device layout [NB, NQS, 2, P, QS] -> [NB, D, S]
    out = np.concatenate(
        [
            r["out"].transpose(0, 2, 3, 1, 4).reshape(NB, D, S)
            for r in res.results
        ],
        axis=0,
    ).astype(np.float32)
    return out, res


def kernel(q, k, v, mask):
    out, _ = run(q, k, v, mask)
    return out
